# revision 1
# baseline (speedup 1.0000x reference)
# Trainium2 Bass kernel for KNN-style sparse cross-attention (v2).
#
# reference semantics (see problem):
#   q  = src @ w_src.T                          [B,S,D]
#   kv = tgt @ w_tgt.T                          [B,S,T,2D]
#   attn[b,h,s,t] = <q[b,s,h], k[b,s,t,h]> / sqrt(DH)
#   softmax over t (with padding mask; fully-masked queries output 0)
#   out = (attn @ v) @ out_proj.T
#
# v2 restructure vs v1: the K projection is algebraically folded away.
#   scores[s,h,t] = src[s] @ (Wq_h^T Wk_h / sqrt(DH)) @ tgt[s,t]^T
# so we precompute M = [Wq_h^T Wk_h / 8]_h on the host ([512, 8*512]),
# compute q_eff = src @ M on device (cheap: 256 queries), and contract
# q_eff directly against the raw tgt tiles per 16-query block with
# q_eff as the PE stationary operand. This removes the K half of the
# kv projection (half the dominant matmul), the gpsimd q*k elementwise
# pass, and K's PSUM->SBUF copies. Softmax runs on full 128-partition
# tiles (rows = (head, query%16)) with a multiplicative mask and is
# compressed back to [8 heads, (s,t)] by a one-hot matmul.
import os
from contextlib import ExitStack

import numpy as np

import concourse.bacc as bacc
import concourse.mybir as mybir
import concourse.tile as tile
from concourse import bass_utils

N_CORES = 8
D = 512          # d_model
H = 8            # heads
DH = 64          # head dim
T = 32           # KNN set size per query
BS = 2048        # B*S total queries
R = BS // N_CORES     # queries per core
RT = R * T            # kv rows per core
PT = 128              # partition tile
KD = D // PT          # 4 contraction tiles over d_model
HT = D // PT          # 4 partition tiles over (h, dh)
QB = 16               # queries per score block (QB*T = 512 psum cols)
NB = R // QB          # 16 score blocks per core

F32 = mybir.dt.float32
F16 = mybir.dt.float16
AX = mybir.AxisListType
ALU = mybir.AluOpType
ACTF = mybir.ActivationFunctionType

W_SUP = 1024          # kv superchunk (cols of tgtT)


def build_program(r=R, t=T, w=W_SUP, n_cores=N_CORES):
    rt = r * t
    nsup = rt // w
    rsup = w // t          # queries per superchunk (32)
    SUB = 512              # matmul moving/psum sub-chunk (one PSUM bank)
    nsub = w // SUB        # score blocks per superchunk (2)
    assert rt % w == 0 and w % t == 0 and w % SUB == 0 and SUB == QB * t

    mdt = F16
    adt = F16

    nc = bacc.Bacc(
        "TRN2",
        target_bir_lowering=False,
        debug=False,
        enable_asserts=False,
        num_devices=n_cores,
    )

    srcT = nc.dram_tensor("srcT", [D, r], mdt, kind="ExternalInput").ap()
    tgtT = nc.dram_tensor("tgtT", [D, rt], mdt, kind="ExternalInput").ap()
    wsT = nc.dram_tensor("wsT", [D, D], mdt, kind="ExternalInput").ap()
    wk2 = nc.dram_tensor("wk2", [PT, (H // 2) * D], mdt, kind="ExternalInput").ap()
    wvT = nc.dram_tensor("wvT", [D, D], mdt, kind="ExternalInput").ap()
    woT = nc.dram_tensor("woT", [D, D], mdt, kind="ExternalInput").ap()
    fmat = nc.dram_tensor("fmat", [H, PT], mdt, kind="ExternalInput").ap()
    cmat = nc.dram_tensor("cmat", [PT, H], mdt, kind="ExternalInput").ap()
    mf = nc.dram_tensor("mf", [PT, rt], mdt, kind="ExternalInput").ap()
    outT = nc.dram_tensor("outT", [D, r], F32, kind="ExternalOutput").ap()

    lp = nc.allow_low_precision("fp32 PSUM accumulation, 16-bit stores")
    lp.__enter__()
    with tile.TileContext(nc) as tc, ExitStack() as ctx:
        consts = ctx.enter_context(tc.tile_pool(name="consts", bufs=1))
        io = ctx.enter_context(tc.tile_pool(name="io", bufs=3))
        kvs = ctx.enter_context(tc.tile_pool(name="kvs", bufs=2))
        one = ctx.enter_context(tc.tile_pool(name="one", bufs=1))
        work = ctx.enter_context(tc.tile_pool(name="work", bufs=2))
        ps_s = ctx.enter_context(tc.tile_pool(name="ps_s", bufs=2, space="PSUM"))
        ps_v = ctx.enter_context(tc.tile_pool(name="ps_v", bufs=2, space="PSUM"))
        ps_bc = ctx.enter_context(tc.tile_pool(name="ps_bc", bufs=2, space="PSUM"))
        ps_p = ctx.enter_context(tc.tile_pool(name="ps_p", bufs=2, space="PSUM"))

        # ---- prefetch V weights + first two tgt superchunks ----
        wv_sb = consts.tile([PT, KD * D], mdt, name="wv_sb")
        for j in range(KD):
            nc.sync.dma_start(
                wv_sb[:, j * D : (j + 1) * D], wvT[j * PT : (j + 1) * PT, :]
            )
        tgs = {}
        for sc in range(min(2, nsup)):
            tg = io.tile([PT, KD * w], mdt, name="tg")
            for j in range(KD):
                nc.sync.dma_start(
                    tg[:, j * w : (j + 1) * w],
                    tgtT[j * PT : (j + 1) * PT, sc * w : (sc + 1) * w],
                )
            tgs[sc] = tg

        # ---- remaining constants ----
        src_sb = consts.tile([PT, KD * r], mdt, name="src_sb")
        nc.sync.dma_start(
            src_sb.rearrange("p (j m) -> p j m", j=KD),
            srcT.rearrange("(j p) m -> p j m", p=PT),
        )
        ws_sb = consts.tile([PT, KD * D], mdt, name="ws_sb")
        nc.sync.dma_start(
            ws_sb.rearrange("p (j m) -> p j m", j=KD),
            wsT.rearrange("(j p) m -> p j m", p=PT),
        )
        wk_sb = consts.tile([PT, (H // 2) * D], mdt, name="wk_sb")
        nc.sync.dma_start(wk_sb, wk2)
        fm_sb = consts.tile([H, PT], mdt, name="fm_sb")
        nc.sync.dma_start(fm_sb, fmat)
        cm_sb = consts.tile([PT, H], mdt, name="cm_sb")
        nc.sync.dma_start(cm_sb, cmat)
        mf_sb = consts.tile([PT, rt], mdt, name="mf_sb")
        nc.sync.dma_start(mf_sb, mf)
        wo_sb = consts.tile([PT, HT * D], mdt, name="wo_sb")
        nc.sync.dma_start(
            wo_sb.rearrange("p (j m) -> p j m", j=HT),
            woT.rearrange("(j p) m -> p j m", p=PT),
        )

        # q_eff laid out so each (d-chunk j, query-block bg) stationary is a
        # contiguous 128-col slice: col = j*(NB*128) + bg*128 + h*16 + s'
        qeff = one.tile([PT, KD * NB * H * QB], adt, name="qeff")
        qeff5 = qeff.rearrange("p (j b h s) -> p j b h s", j=KD, b=NB, h=H)

        q_sb = one.tile([PT, HT * r], adt, name="q_sb")

        def qeff_stage():
            for m in range(HT):
                qpp = ps_bc.tile([PT, r], F32, name="qpp", tag="bc")
                for j in range(KD):
                    nc.tensor.matmul(
                        qpp,
                        ws_sb[:, j * D + m * PT : j * D + (m + 1) * PT],
                        src_sb[:, j * r : (j + 1) * r],
                        start=(j == 0),
                        stop=(j == KD - 1),
                    )
                nc.scalar.copy(q_sb[:, m * r : (m + 1) * r], qpp)
            for h in range(H):
                p0 = (h % 2) * 64
                for c in range(KD):
                    qp = ps_bc.tile([PT, r], F32, name="qp", tag="bc")
                    nc.tensor.matmul(
                        qp,
                        wk_sb[p0 : p0 + 64,
                              (h // 2) * D + c * PT : (h // 2) * D + (c + 1) * PT],
                        q_sb[p0 : p0 + 64, (h // 2) * r : (h // 2 + 1) * r],
                        start=True,
                        stop=True,
                    )
                    nc.scalar.copy(
                        qeff5[:, c, :, h, :],
                        qp.rearrange("p (b s) -> p b s", s=QB),
                    )

        def vproj_stage(sc, vT, half):
            """V projection for superchunk sc (halves 0/1: m in 0-1 / 2-3)."""
            tg = tgs[sc]
            for m in (0, 1) if half == 0 else (2, 3):
                for s in range(nsub):
                    pv = ps_v.tile([PT, SUB], F32, name="pv")
                    for j in range(KD):
                        nc.tensor.matmul(
                            pv,
                            wv_sb[:, j * D + m * PT : j * D + (m + 1) * PT],
                            tg[:, j * w + s * SUB : j * w + (s + 1) * SUB],
                            start=(j == 0),
                            stop=(j == KD - 1),
                        )
                    nc.scalar.copy(
                        vT[:, m * w + s * SUB : m * w + (s + 1) * SUB], pv
                    )

        def qk_stage(sc):
            """scores for the two 16-query blocks of superchunk sc."""
            tg = tgs[sc]
            sps = []
            for b in range(nsub):
                bg = sc * nsub + b         # global block id
                sp = ps_s.tile([PT, SUB], F32, name="sp")
                for j in range(KD):
                    base = (j * NB + bg) * (H * QB)
                    nc.tensor.matmul(
                        sp,
                        qeff[:, base : base + H * QB],
                        tg[:, j * w + b * SUB : j * w + (b + 1) * SUB],
                        start=(j == 0),
                        stop=(j == KD - 1),
                    )
                sps.append(sp)
            return sps

        def softmax_stage(sc, sps):
            """masked softmax on [128,(QB,t)] tiles -> normalized A tiles."""
            As = []
            for b in range(nsub):
                bg = sc * nsub + b
                eb = work.tile([PT, SUB], adt, name="eb")
                nc.scalar.activation(eb, sps[b], ACTF.Exp)
                em = work.tile([PT, SUB], adt, name="em")
                nc.gpsimd.tensor_mul(
                    em, eb, mf_sb[:, bg * SUB : (bg + 1) * SUB]
                )
                sums = work.tile([PT, QB], F32, name="sums")
                nc.vector.reduce_sum(
                    sums, em.rearrange("p (s t) -> p s t", t=t), axis=AX.X
                )
                nc.vector.tensor_scalar_add(sums, sums, 1e-20)
                rec = work.tile([PT, QB], F32, name="rec")
                nc.vector.reciprocal(rec, sums)
                ab = work.tile([PT, SUB], adt, name="ab")
                nc.gpsimd.tensor_mul(
                    ab.rearrange("p (s t) -> p s t", t=t),
                    em.rearrange("p (s t) -> p s t", t=t),
                    rec.unsqueeze(2).broadcast_to([PT, QB, t]),
                )
                As.append(ab)
            return As

        def compress_stage(sc, As, attn):
            for b in range(nsub):
                pb = ps_p.tile([H, SUB], F32, name="pb")
                nc.tensor.matmul(pb, cm_sb, As[b], start=True, stop=True)
                nc.scalar.copy(attn[:, b * SUB : (b + 1) * SUB], pb)

        oav = one.tile([PT, HT * r], adt, name="oav")

        def av_stage(sc, vT, attn, tail=False):
            """shared attn broadcast (v cols are head-uniform permuted),
            mult + t-reduce on vector. In the drain tail (no vproj work
            left), stage bc to fp16 via the idle scalar engine and split
            the mults vector/gpsimd so vector only paces the reduces."""
            uts = [work.tile([PT, w], adt, name="ut", bufs=4) for _ in range(HT)]
            for s in range(nsub):
                bc = ps_bc.tile([PT, SUB], F32, name="bc", tag="bc")
                nc.tensor.matmul(
                    bc,
                    fm_sb,
                    attn[:, s * SUB : (s + 1) * SUB],
                    start=True,
                    stop=True,
                )
                if tail:
                    bch = work.tile([PT, SUB], adt, name="bch", bufs=3)
                    nc.scalar.copy(bch, bc)
                for j in range(HT):
                    eng = nc.gpsimd if tail and j >= 2 else nc.vector
                    eng.tensor_mul(
                        uts[j][:, s * SUB : (s + 1) * SUB],
                        bch if tail else bc,
                        vT[:, j * w + s * SUB : j * w + (s + 1) * SUB],
                    )
            for j in range(HT):
                nc.vector.reduce_sum(
                    oav[:, j * r + sc * rsup : j * r + (sc + 1) * rsup],
                    uts[j].rearrange("p (r t) -> p r t", t=t),
                    axis=AX.X,
                )

        # ---- main pipeline ----
        # v-proj runs two superchunks ahead; the attention chain
        # (qk -> softmax -> compress) runs one superchunk ahead of the
        # av stage that consumes it, so avbc never waits on softmax.
        vTs = {}
        attns = {}
        vT0 = kvs.tile([PT, HT * w], adt, name="vT", bufs=3)
        vproj_stage(0, vT0, 0)
        vproj_stage(0, vT0, 1)
        vTs[0] = vT0
        qeff_stage()
        if nsup > 1:
            vT1 = kvs.tile([PT, HT * w], adt, name="vT", bufs=3)
            vproj_stage(1, vT1, 0)
            vproj_stage(1, vT1, 1)
            vTs[1] = vT1
        sps = qk_stage(0)
        As = softmax_stage(0, sps)
        attn0 = work.tile([H, w], adt, name="attn", bufs=3)
        compress_stage(0, As, attn0)
        attns[0] = attn0

        for sc in range(nsup):
            pre = sc + 1
            if pre < nsup:
                sps = qk_stage(pre)
                As = softmax_stage(pre, sps)
            nxt = sc + 2
            if nxt < nsup:
                tg = io.tile([PT, KD * w], mdt, name="tg")
                for j in range(KD):
                    nc.sync.dma_start(
                        tg[:, j * w : (j + 1) * w],
                        tgtT[j * PT : (j + 1) * PT, nxt * w : (nxt + 1) * w],
                    )
                tgs[nxt] = tg
                vTn = kvs.tile([PT, HT * w], adt, name="vT", bufs=3)
                vproj_stage(nxt, vTn, 0)
            if pre < nsup:
                attn = work.tile([H, w], adt, name="attn", bufs=3)
                compress_stage(pre, As, attn)
                attns[pre] = attn
            if nxt < nsup:
                vproj_stage(nxt, vTn, 1)
                vTs[nxt] = vTn
            av_stage(sc, vTs[sc], attns[sc], tail=(sc >= nsup - 5))

        # ---- output projection ----
        for e in range(HT):
            op = ps_bc.tile([PT, r], F32, name="op", tag="bc")
            for j in range(HT):
                nc.tensor.matmul(
                    op,
                    wo_sb[:, j * D + e * PT : j * D + (e + 1) * PT],
                    oav[:, j * r : (j + 1) * r],
                    start=(j == 0),
                    stop=(j == HT - 1),
                )
            res = work.tile([PT, r], F32, name="res")
            nc.scalar.copy(res, op)
            nc.sync.dma_start(outT[e * PT : (e + 1) * PT, :], res)

    lp.__exit__(None, None, None)
    nc.compile()
    return nc


_PROGRAM = None


def _get_program():
    global _PROGRAM
    if _PROGRAM is None:
        _PROGRAM = build_program()
    return _PROGRAM


def prep_inputs(src, tgt, tgt_padding_mask, in_proj_weight, in_proj_bias,
                out_proj_weight, out_proj_bias):
    """Host-side shard + layout prep. Returns per-core in_maps."""
    mnp = np.float16
    f32 = np.float32
    src2 = np.asarray(src, dtype=f32).reshape(BS, D)
    tgt2 = np.asarray(tgt, dtype=f32).reshape(BS * T, D)
    mask2 = np.asarray(tgt_padding_mask).astype(bool).reshape(BS, T)
    wm = np.asarray(in_proj_weight, dtype=f32)
    wo = np.asarray(out_proj_weight, dtype=f32)
    Wq, Wk, Wv = wm[:D], wm[D : 2 * D], wm[2 * D :]

    # permute v's output dims so every 128-col tile has head(p) = p//16:
    # new column m*128+p holds old dim (p//16)*64 + m*16 + (p%16)
    mm_, pp_ = np.arange(HT)[:, None], np.arange(PT)[None, :]
    colmap = ((pp_ // QB) * DH + mm_ * QB + (pp_ % QB)).reshape(D)

    scl = f32(1.0 / np.sqrt(DH))
    wsT = np.ascontiguousarray((Wq * scl).T).astype(mnp)
    wk2 = np.ascontiguousarray(
        Wk.reshape(H // 2, 2, DH, D).transpose(1, 2, 0, 3).reshape(PT, (H // 2) * D)
    ).astype(mnp)
    wvT = np.ascontiguousarray(Wv.T[:, colmap]).astype(mnp)
    woT = np.ascontiguousarray(wo.T[colmap, :]).astype(mnp)

    fmat = np.zeros((H, PT), dtype=mnp)
    fmat[np.arange(PT) // QB, np.arange(PT)] = 1.0
    # cmat[(h, s'), h'] = 1 iff h == h'
    cmat = np.zeros((PT, H), dtype=mnp)
    cmat[np.arange(PT), np.arange(PT) // QB] = 1.0

    # mask row pattern: eye16[s', s_local] = 1 iff s_local % 16 == s'
    eye16 = (np.arange(R)[None, :] % QB == np.arange(QB)[:, None])

    in_maps = []
    for c in range(N_CORES):
        rows = slice(c * R, (c + 1) * R)
        kvrows = slice(c * RT, (c + 1) * RT)
        valid = ~mask2[rows]                       # [R, T] True = usable
        mfc = (eye16[:, :, None] & valid[None]).reshape(QB, RT)
        mfc = np.broadcast_to(mfc[None], (H, QB, RT)).reshape(PT, RT)
        in_maps.append({
            "srcT": np.ascontiguousarray(src2[rows].T.astype(mnp)),
            "tgtT": np.ascontiguousarray(tgt2[kvrows].T.astype(mnp)),
            "wsT": wsT, "wk2": wk2, "wvT": wvT, "woT": woT,
            "fmat": fmat, "cmat": cmat,
            "mf": np.ascontiguousarray(mfc.astype(mnp)),
        })
    return in_maps


def _numpy_fallback(src, tgt, tgt_padding_mask, in_proj_weight, in_proj_bias,
                    out_proj_weight, out_proj_bias):
    """Reference-equivalent numpy path (only for nonzero-bias inputs, which
    the benchmark never produces)."""
    B, S, _ = src.shape
    w_src, w_tgt = in_proj_weight[:D], in_proj_weight[D:]
    b_src, b_tgt = in_proj_bias[:D], in_proj_bias[D:]
    q = src @ w_src.T + b_src
    kv = tgt @ w_tgt.T + b_tgt
    k, v = kv[..., :D], kv[..., D:]
    inv = tgt_padding_mask.astype(bool)
    noval = inv.all(-1)
    inv = inv & ~noval[..., None]
    q = q.reshape(B, S, H, DH)
    k = k.reshape(B, S, T, H, DH)
    v = v.reshape(B, S, T, H, DH)
    att = np.einsum("bshd,bsthd->bhst", q, k)
    att = np.where(inv[:, None], -np.inf, att) / np.sqrt(DH)
    att = att - att.max(-1, keepdims=True)
    att = np.exp(att)
    att = att / att.sum(-1, keepdims=True)
    out = np.einsum("bhst,bsthd->bshd", att, v).reshape(B, S, D)
    out = out @ out_proj_weight.T + out_proj_bias
    return np.where(noval[..., None], 0.0, out).astype(np.float32)


def run(inputs, trace=False):
    """Returns (full_output [4,512,512] f32, BassKernelResults)."""
    in_maps = prep_inputs(**inputs)
    nc = _get_program()
    res = bass_utils.run_bass_kernel_spmd(
        nc, in_maps, core_ids=list(range(N_CORES)), trace=trace
    )
    out = np.empty((BS, D), dtype=np.float32)
    for c in range(N_CORES):
        out[c * R : (c + 1) * R] = res.results[c]["outT"].T
    return out.reshape(4, 512, D), res


def kernel(**inputs):
    inputs = {k: np.asarray(v) for k, v in inputs.items()}
    if (np.any(inputs["in_proj_bias"]) or np.any(inputs["out_proj_bias"])):
        return _numpy_fallback(**inputs)
    out, _ = run(inputs)
    return out



# revision 14
# speedup vs baseline: 1.2073x; 1.2073x over previous
# Trainium2 Bass kernel for KNN-style sparse cross-attention (v4).
#
# reference semantics:
#   q  = src @ w_src.T                          [B,S,D]
#   kv = tgt @ w_tgt.T                          [B,S,T,2D]
#   attn[b,h,s,t] = <q[b,s,h], k[b,s,t,h]> / sqrt(DH)
#   softmax over t (with padding mask; fully-masked queries output 0)
#   out = (attn @ v) @ out_proj.T
#
# Structure (v3/v4): contract over t BEFORE projecting with Wv.
#   q_eff = src Wq^T Wk / sqrt(DH)   (K folded away, rank-64 two-stage)
#   scores computed TRANSPOSED: [st, (g,h,q')] tiles, st = kv-row on
#     partitions, 4 queries x 32 t per 128-row group, 32-col matmuls.
#   softmax: exp on scalar; per-group column sums via one-hot-stationary
#     matmuls (padding mask folded into the host-built stationary);
#     cross-query garbage killed by accumulating +BIG into the wrong
#     (row-group, query) sums, so 1/sum ~ 0 there; 1/x = Exp(-Ln(x))
#     on scalar; reciprocal broadcast back over partitions by a 4-row
#     one-hot matmul.  Invalid kv rows are ZEROED in tgn on the host so
#     they contribute nothing to ctx (also handles fully-masked queries).
#   ctx[d, (g,h,q')] = A^T tgt   (tgt natural-layout stationary)
#   out_av = ctx @ Wv_h^T per head (only 2048 ctx rows projected)
#   out = out_av @ Wo^T  (flipped: out_av stationary -> out is [s, d'])
import os
from contextlib import ExitStack

import numpy as np

import concourse.bacc as bacc
import concourse.mybir as mybir
import concourse.tile as tile
from concourse import bass_utils

N_CORES = 8
D = 512          # d_model
H = 8            # heads
DH = 64          # head dim
T = 32           # KNN set size per query
BS = 2048        # B*S total queries
R = BS // N_CORES     # queries per core (256)
RT = R * T            # kv rows per core (8192)
PT = 128              # partition tile
KD = D // PT          # 4 contraction tiles over d_model
QB = 16               # queries per block
NB = R // QB          # 16 blocks per core
G = 4                 # query groups per block (4 queries x 32 t = 128 st)
W = 1024              # kv rows per superchunk (2 blocks)
NSUP = RT // W        # 8 superchunks
BIG = 60000.0         # garbage-sum offset (f16-representable)

F32 = mybir.dt.float32
F16 = mybir.dt.float16
AX = mybir.AxisListType
ACTF = mybir.ActivationFunctionType


def build_program(n_cores=N_CORES):
    mdt = F16

    nc = bacc.Bacc(
        "TRN2",
        target_bir_lowering=False,
        debug=False,
        enable_asserts=False,
        num_devices=n_cores,
    )

    srcT = nc.dram_tensor("srcT", [D, R], mdt, kind="ExternalInput").ap()
    tgtT = nc.dram_tensor("tgtT", [D, RT], mdt, kind="ExternalInput").ap()
    tgn = nc.dram_tensor("tgn", [RT, D], mdt, kind="ExternalInput").ap()
    wsT = nc.dram_tensor("wsT", [D, D], mdt, kind="ExternalInput").ap()
    wk2 = nc.dram_tensor("wk2", [PT, (H // 2) * D], mdt, kind="ExternalInput").ap()
    wvT = nc.dram_tensor("wvT", [D, D], mdt, kind="ExternalInput").ap()
    won = nc.dram_tensor("won", [D, D], mdt, kind="ExternalInput").ap()
    # aux consts: [4, 260] = b4f [4,128] | oh4 [4,128] | anti4 [4,4]
    aux = nc.dram_tensor("aux", [4, 260], mdt, kind="ExternalInput").ap()
    # per-(block, g) sums stationaries with padding mask folded in:
    # dsb[p, (blk, g, k)] = [p//32 == k] * valid(st row p of group g)
    dsb = nc.dram_tensor("dsb", [PT, NB * G * 4], mdt, kind="ExternalInput").ap()
    outn = nc.dram_tensor("outn", [R, D], F32, kind="ExternalOutput").ap()

    lp = nc.allow_low_precision("fp32 PSUM accumulation, 16-bit stores")
    lp.__enter__()
    with tile.TileContext(nc) as tc, ExitStack() as ctx:
        consts = ctx.enter_context(tc.tile_pool(name="consts", bufs=1))
        io_t = ctx.enter_context(tc.tile_pool(name="io_t", bufs=3))
        io_n = ctx.enter_context(tc.tile_pool(name="io_n", bufs=3))
        one = ctx.enter_context(tc.tile_pool(name="one", bufs=1))
        blkp = ctx.enter_context(tc.tile_pool(name="blkp", bufs=4))
        work = ctx.enter_context(tc.tile_pool(name="work", bufs=2))
        ps_blk = ctx.enter_context(tc.tile_pool(name="ps_blk", bufs=3, space="PSUM"))
        ps_ctx = ctx.enter_context(tc.tile_pool(name="ps_ctx", bufs=3, space="PSUM"))
        ps_bc = ctx.enter_context(tc.tile_pool(name="ps_bc", bufs=2, space="PSUM"))

        # ---- phase 1 DMAs: what qeff needs, plus tiny consts ----
        aux_sb = consts.tile([4, 260], mdt, name="aux_sb")
        nc.sync.dma_start(aux_sb, aux)
        b4f = aux_sb[:, 0:128]
        oh4 = aux_sb[:, 128:256]
        anti4 = aux_sb[:, 256:260]
        ds_sb = consts.tile([PT, NB * G * 4], mdt, name="ds_sb")
        nc.sync.dma_start(ds_sb, dsb)
        eps_sb = consts.tile([4, 1], F32, name="eps_sb")
        nc.gpsimd.memset(eps_sb, 1e-4)
        neg_sb = consts.tile([4, 1], F32, name="neg_sb")
        nc.gpsimd.memset(neg_sb, -1.0)
        src_sb = consts.tile([PT, KD * R], mdt, name="src_sb")
        nc.sync.dma_start(
            src_sb.rearrange("p (j m) -> p j m", j=KD),
            srcT.rearrange("(j p) m -> p j m", p=PT),
        )
        ws_sb = consts.tile([PT, KD * D], mdt, name="ws_sb")
        nc.sync.dma_start(
            ws_sb.rearrange("p (j m) -> p j m", j=KD),
            wsT.rearrange("(j p) m -> p j m", p=PT),
        )
        wk_sb = consts.tile([PT, (H // 2) * D], mdt, name="wk_sb")
        nc.sync.dma_start(wk_sb, wk2)

        # ---- phase 2: tgt superchunk prefetch ----
        tgTs = {}
        tgNs = {}

        def fetch_sup(sc):
            tgT = io_t.tile([PT, KD * W], mdt, name="tgT")
            nc.sync.dma_start(
                tgT.rearrange("p (j m) -> p j m", j=KD),
                tgtT.rearrange("(j p) m -> p j m", p=PT)[:, :, sc * W : (sc + 1) * W],
            )
            tgTs[sc] = tgT
            tgN = io_n.tile([PT, (W // PT) * D], mdt, name="tgN")
            nc.sync.dma_start(
                tgN.rearrange("p (c d) -> p c d", c=W // PT),
                tgn.rearrange("(s c p) d -> s p c d", p=PT, c=W // PT)[sc],
            )
            tgNs[sc] = tgN

        for sc in range(min(3, NSUP)):
            fetch_sup(sc)

        # ---- phase 3: tail weights ----
        wv_sb = consts.tile([PT, KD * D], mdt, name="wv_sb")
        nc.sync.dma_start(
            wv_sb.rearrange("p (j m) -> p j m", j=KD),
            wvT.rearrange("(j p) m -> p j m", p=PT),
        )
        wo_sb = consts.tile([PT, KD * D], mdt, name="wo_sb")
        nc.sync.dma_start(
            wo_sb.rearrange("p (j m) -> p j m", j=KD),
            won.rearrange("(j p) m -> p j m", p=PT),
        )

        # ---- q_eff: [128, j(4) x h(8) x s(256)]; the (h,q') gather for
        # QK happens in the matmul moving AP, copies stay contiguous.
        qeff = one.tile([PT, KD * H * R], mdt, name="qeff")
        qeff4 = qeff.rearrange("p (j h s) -> p j h s", j=KD, h=H)
        q_sb = one.tile([PT, KD * R], mdt, name="q_sb")

        def qeff_stage():
            for m in range(KD):
                qpp = ps_bc.tile([PT, R], F32, name="qpp", tag="bc")
                for j in range(KD):
                    nc.tensor.matmul(
                        qpp,
                        ws_sb[:, j * D + m * PT : j * D + (m + 1) * PT],
                        src_sb[:, j * R : (j + 1) * R],
                        start=(j == 0),
                        stop=(j == KD - 1),
                    )
                (nc.scalar.copy if m % 2 == 0 else nc.vector.tensor_copy)(
                    q_sb[:, m * R : (m + 1) * R], qpp
                )
            for h in range(H):
                p0 = (h % 2) * 64
                for c in range(KD):
                    qp = ps_bc.tile([PT, R], F32, name="qp", tag="bc")
                    nc.tensor.matmul(
                        qp,
                        wk_sb[p0 : p0 + 64,
                              (h // 2) * D + c * PT : (h // 2) * D + (c + 1) * PT],
                        q_sb[p0 : p0 + 64, (h // 2) * R : (h // 2 + 1) * R],
                        start=True,
                        stop=True,
                    )
                    (nc.scalar.copy if (h * KD + c) % 2 == 0
                     else nc.vector.tensor_copy)(qeff4[:, c, h, :], qp)

        qeff_stage()

        # ctx layout: [p, j(4) x blk(16) x (g,h,q')(128)]
        ctx_sb = one.tile([PT, KD * NB * PT], mdt, name="ctx_sb")
        ctx5 = ctx_sb.rearrange(
            "p (j b g h q) -> p j b g h q", j=KD, b=NB, g=G, h=H
        )
        oav_sb = one.tile([PT, KD * R], mdt, name="oav_sb")

        def do_block(blk):
            sc = blk // 2
            bl = blk % 2
            tgT = tgTs[sc].rearrange("p (j m) -> p j m", j=KD)
            tgN = tgNs[sc].rearrange("p (c d) -> p c d", c=W // PT)
            bp = ps_blk.tile([PT, 384], F32, name="bp")
            scr = bp[:, 0:128]
            sums = bp[0:4, 128:256]
            rb = bp[:, 256:384]
            for g in range(G):
                # moving: qeff[(j), h(8), q'(4)] gathered via AP
                for j in range(KD):
                    nc.tensor.matmul(
                        scr[:, g * 32 : (g + 1) * 32],
                        tgT[:, j, bl * 512 + g * PT : bl * 512 + (g + 1) * PT],
                        qeff4[:, j, :, blk * QB + g * 4 : blk * QB + g * 4 + 4],
                        start=(j == 0),
                        stop=(j == KD - 1),
                    )
            em = blkp.tile([PT, PT], mdt, name="em")
            nc.scalar.activation(em, scr, ACTF.Exp)
            # sums[k, (g,h,q')]: anti-BIG first, then per-g masked sums
            nc.tensor.matmul(sums, anti4, b4f, start=True, stop=False,
                             skip_group_check=True)
            for g in range(G):
                nc.tensor.matmul(
                    sums[:, g * 32 : (g + 1) * 32],
                    ds_sb[:, (blk * G + g) * 4 : (blk * G + g) * 4 + 4],
                    em[:, g * 32 : (g + 1) * 32],
                    start=False,
                    stop=True,
                    skip_group_check=True,
                )
            lns = blkp.tile([4, PT], F32, name="lns")
            nc.scalar.activation(lns, sums, ACTF.Ln, bias=eps_sb)
            rc4 = blkp.tile([4, PT], mdt, name="rc4")
            nc.scalar.activation(rc4, lns, ACTF.Exp, scale=neg_sb)
            nc.tensor.matmul(rb, oh4, rc4, start=True, stop=True)
            ab = blkp.tile([PT, PT], mdt, name="ab")
            nc.vector.tensor_mul(ab, em, rb)
            cp = ps_ctx.tile([PT, 512], F32, name="cp")
            for dc in range(KD):
                for g in range(G):
                    nc.tensor.matmul(
                        cp[:, dc * PT + g * 32 : dc * PT + (g + 1) * 32],
                        tgN[:, bl * G + g, dc * PT : (dc + 1) * PT],
                        ab[:, g * 32 : (g + 1) * 32],
                        start=True,
                        stop=True,
                    )
            for dc in range(KD):
                eng = (nc.scalar.copy, nc.vector.tensor_copy,
                       nc.vector.tensor_copy, nc.scalar.copy)[dc]
                eng(
                    ctx_sb[:, (dc * NB + blk) * PT : (dc * NB + blk + 1) * PT],
                    cp[:, dc * PT : (dc + 1) * PT],
                )

        def do_tail(half):
            # project ctx -> out_av for 128 queries (8 blocks), then out.
            for h in range(H):
                ovp = ps_bc.tile([64, PT], F32, name="ovp", tag="bc")
                for dc in range(KD):
                    mov = ctx5[:, dc, half * 8 : (half + 1) * 8, :, h, :]
                    nc.tensor.matmul(
                        ovp,
                        wv_sb[:, dc * D + h * DH : dc * D + (h + 1) * DH],
                        mov,
                        start=(dc == 0),
                        stop=(dc == KD - 1),
                    )
                p0 = (h % 2) * 64
                (nc.scalar.copy if h % 2 == 0 else nc.vector.tensor_copy)(
                    oav_sb[p0 : p0 + 64,
                           (h // 2) * R + half * PT : (h // 2) * R + (half + 1) * PT],
                    ovp,
                )
            op = ps_bc.tile([PT, D], F32, name="op", tag="bc")
            for hh in range(KD):
                nc.tensor.matmul(
                    op,
                    oav_sb[:, hh * R + half * PT : hh * R + (half + 1) * PT],
                    wo_sb[:, hh * D : (hh + 1) * D],
                    start=(hh == 0),
                    stop=(hh == KD - 1),
                )
            res = work.tile([PT, D], F32, name="res")
            nc.scalar.copy(res, op)
            nc.sync.dma_start(outn[half * PT : (half + 1) * PT, :], res)

        for blk in range(NB):
            if blk % 2 == 0:
                nxt = blk // 2 + 3
                if nxt < NSUP:
                    fetch_sup(nxt)
            do_block(blk)
            if blk == 7:
                do_tail(0)
        do_tail(1)

    lp.__exit__(None, None, None)
    nc.compile()
    return nc


_PROGRAM = None


def _get_program():
    global _PROGRAM
    if _PROGRAM is None:
        _PROGRAM = build_program()
    return _PROGRAM


def prep_inputs(src, tgt, tgt_padding_mask, in_proj_weight, in_proj_bias,
                out_proj_weight, out_proj_bias):
    """Host-side shard + layout prep. Returns per-core in_maps."""
    mnp = np.float16
    f32 = np.float32
    src2 = np.asarray(src, dtype=f32).reshape(BS, D)
    tgt2 = np.asarray(tgt, dtype=f32).reshape(BS * T, D)
    mask2 = np.asarray(tgt_padding_mask).astype(bool).reshape(BS, T)
    wm = np.asarray(in_proj_weight, dtype=f32)
    wo = np.asarray(out_proj_weight, dtype=f32)
    Wq, Wk, Wv = wm[:D], wm[D : 2 * D], wm[2 * D :]

    scl = f32(1.0 / np.sqrt(DH))
    wsT = np.ascontiguousarray((Wq * scl).T).astype(mnp)
    wk2 = np.ascontiguousarray(
        Wk.reshape(H // 2, 2, DH, D).transpose(1, 2, 0, 3).reshape(PT, (H // 2) * D)
    ).astype(mnp)
    wvT = np.ascontiguousarray(Wv.T).astype(mnp)
    won = np.ascontiguousarray(wo.T).astype(mnp)

    # aux consts
    aux = np.zeros((4, 260), dtype=mnp)
    cc = np.arange(PT)
    aux[:, 0:128] = (cc[None, :] % 4 == np.arange(4)[:, None])      # b4f
    aux[:, 128:256] = (cc[None, :] // 32 == np.arange(4)[:, None])  # oh4
    aux[:, 256:260] = BIG * (1.0 - np.eye(4, dtype=f32))            # anti4

    valid_all = ~mask2                                              # [BS, T]
    tgt16 = tgt2.astype(mnp)
    tgt16[~valid_all.reshape(-1)] = 0                               # zero invalid kv rows

    pp = np.arange(PT)
    in_maps = []
    for c in range(N_CORES):
        rows = slice(c * R, (c + 1) * R)
        kvrows = slice(c * RT, (c + 1) * RT)
        valid = valid_all[rows]                                     # [R, T]
        # dsb[p, blk, g, k] = [p//32==k] * valid[blk*16+g*4+p//32, p%32]
        vg = valid.reshape(NB, G, 4, T)                             # [blk,g,q'',t]
        dsbm = np.zeros((PT, NB, G, 4), dtype=mnp)
        for k in range(4):
            sel = pp // 32 == k
            dsbm[sel, :, :, k] = vg[:, :, k, :].transpose(2, 0, 1)[pp[sel] % 32 - 0]
        in_maps.append({
            "srcT": np.ascontiguousarray(src2[rows].T.astype(mnp)),
            "tgtT": np.ascontiguousarray(tgt2[kvrows].T.astype(mnp)),
            "tgn": np.ascontiguousarray(tgt16[kvrows]),
            "wsT": wsT, "wk2": wk2, "wvT": wvT, "won": won,
            "aux": aux,
            "dsb": np.ascontiguousarray(dsbm.reshape(PT, NB * G * 4)),
        })
    return in_maps


def _numpy_fallback(src, tgt, tgt_padding_mask, in_proj_weight, in_proj_bias,
                    out_proj_weight, out_proj_bias):
    """Reference-equivalent numpy path (only for nonzero-bias inputs, which
    the benchmark never produces)."""
    B, S, _ = src.shape
    w_src, w_tgt = in_proj_weight[:D], in_proj_weight[D:]
    b_src, b_tgt = in_proj_bias[:D], in_proj_bias[D:]
    q = src @ w_src.T + b_src
    kv = tgt @ w_tgt.T + b_tgt
    k, v = kv[..., :D], kv[..., D:]
    inv = tgt_padding_mask.astype(bool)
    noval = inv.all(-1)
    inv = inv & ~noval[..., None]
    q = q.reshape(B, S, H, DH)
    k = k.reshape(B, S, T, H, DH)
    v = v.reshape(B, S, T, H, DH)
    att = np.einsum("bshd,bsthd->bhst", q, k)
    att = np.where(inv[:, None], -np.inf, att) / np.sqrt(DH)
    att = att - att.max(-1, keepdims=True)
    att = np.exp(att)
    att = att / att.sum(-1, keepdims=True)
    out = np.einsum("bhst,bsthd->bshd", att, v).reshape(B, S, D)
    out = out @ out_proj_weight.T + out_proj_bias
    return np.where(noval[..., None], 0.0, out).astype(np.float32)


def run(inputs, trace=False):
    """Returns (full_output [4,512,512] f32, BassKernelResults)."""
    in_maps = prep_inputs(**inputs)
    nc = _get_program()
    res = bass_utils.run_bass_kernel_spmd(
        nc, in_maps, core_ids=list(range(N_CORES)), trace=trace
    )
    out = np.empty((BS, D), dtype=np.float32)
    for c in range(N_CORES):
        out[c * R : (c + 1) * R] = res.results[c]["outn"]
    return out.reshape(4, 512, D), res


def kernel(**inputs):
    inputs = {k: np.asarray(v) for k, v in inputs.items()}
    if (np.any(inputs["in_proj_bias"]) or np.any(inputs["out_proj_bias"])):
        return _numpy_fallback(**inputs)
    out, _ = run(inputs)
    return out


# revision 20
# speedup vs baseline: 1.2733x; 1.0547x over previous
# Trainium2 Bass kernel for KNN-style sparse cross-attention (v4).
#
# reference semantics:
#   q  = src @ w_src.T                          [B,S,D]
#   kv = tgt @ w_tgt.T                          [B,S,T,2D]
#   attn[b,h,s,t] = <q[b,s,h], k[b,s,t,h]> / sqrt(DH)
#   softmax over t (with padding mask; fully-masked queries output 0)
#   out = (attn @ v) @ out_proj.T
#
# Structure (v3/v4): contract over t BEFORE projecting with Wv.
#   q_eff = src Wq^T Wk / sqrt(DH)   (K folded away, rank-64 two-stage)
#   scores computed TRANSPOSED: [st, (g,h,q')] tiles, st = kv-row on
#     partitions, 4 queries x 32 t per 128-row group, 32-col matmuls.
#   softmax: exp on scalar; per-group column sums via one-hot-stationary
#     matmuls (padding mask folded into the host-built stationary);
#     cross-query garbage killed by accumulating +BIG into the wrong
#     (row-group, query) sums, so 1/sum ~ 0 there; 1/x = Exp(-Ln(x))
#     on scalar; reciprocal broadcast back over partitions by a 4-row
#     one-hot matmul.  Invalid kv rows are ZEROED in tgn on the host so
#     they contribute nothing to ctx (also handles fully-masked queries).
#   ctx[d, (g,h,q')] = A^T tgt   (tgt natural-layout stationary)
#   out_av = ctx @ Wv_h^T per head (only 2048 ctx rows projected)
#   out = out_av @ Wo^T  (flipped: out_av stationary -> out is [s, d'])
import os
from contextlib import ExitStack

import numpy as np

import concourse.bacc as bacc
import concourse.mybir as mybir
import concourse.tile as tile
from concourse import bass_utils

N_CORES = 8
D = 512          # d_model
H = 8            # heads
DH = 64          # head dim
T = 32           # KNN set size per query
BS = 2048        # B*S total queries
R = BS // N_CORES     # queries per core (256)
RT = R * T            # kv rows per core (8192)
PT = 128              # partition tile
KD = D // PT          # 4 contraction tiles over d_model
QB = 16               # queries per block
NB = R // QB          # 16 blocks per core
G = 4                 # query groups per block (4 queries x 32 t = 128 st)
W = 1024              # kv rows per superchunk (2 blocks)
NSUP = RT // W        # 8 superchunks
BIG = 60000.0         # garbage-sum offset (f16-representable)

F32 = mybir.dt.float32
F16 = mybir.dt.float16
AX = mybir.AxisListType
ACTF = mybir.ActivationFunctionType


def build_program(n_cores=N_CORES):
    mdt = F16

    nc = bacc.Bacc(
        "TRN2",
        target_bir_lowering=False,
        debug=False,
        enable_asserts=False,
        num_devices=n_cores,
    )

    srcT = nc.dram_tensor("srcT", [D, R], mdt, kind="ExternalInput").ap()
    tgtT = nc.dram_tensor("tgtT", [D, RT], mdt, kind="ExternalInput").ap()
    tgn = nc.dram_tensor("tgn", [RT, D], mdt, kind="ExternalInput").ap()
    wsT = nc.dram_tensor("wsT", [D, D], mdt, kind="ExternalInput").ap()
    wk2 = nc.dram_tensor("wk2", [PT, (H // 2) * D], mdt, kind="ExternalInput").ap()
    wvT = nc.dram_tensor("wvT", [D, D], mdt, kind="ExternalInput").ap()
    won = nc.dram_tensor("won", [D, D], mdt, kind="ExternalInput").ap()
    # aux consts: [4, 260] = b4f [4,128] | oh4 [4,128] | anti4 [4,4]
    aux = nc.dram_tensor("aux", [4, 260], mdt, kind="ExternalInput").ap()
    # per-(block, g) sums stationaries with padding mask folded in:
    # dsb[p, (blk, g, k)] = [p//32 == k] * valid(st row p of group g)
    dsb = nc.dram_tensor("dsb", [PT, NB * G * 4], mdt, kind="ExternalInput").ap()
    outn = nc.dram_tensor("outn", [R, D], F32, kind="ExternalOutput").ap()

    lp = nc.allow_low_precision("fp32 PSUM accumulation, 16-bit stores")
    lp.__enter__()
    with tile.TileContext(nc) as tc, ExitStack() as ctx:
        consts = ctx.enter_context(tc.tile_pool(name="consts", bufs=1))
        io_t = ctx.enter_context(tc.tile_pool(name="io_t", bufs=3))
        io_n = ctx.enter_context(tc.tile_pool(name="io_n", bufs=3))
        one = ctx.enter_context(tc.tile_pool(name="one", bufs=1))
        blkp = ctx.enter_context(tc.tile_pool(name="blkp", bufs=4))
        work = ctx.enter_context(tc.tile_pool(name="work", bufs=2))
        ps_blk = ctx.enter_context(tc.tile_pool(name="ps_blk", bufs=2, space="PSUM"))
        ps_ctx = ctx.enter_context(tc.tile_pool(name="ps_ctx", bufs=2, space="PSUM"))
        ps_bc = ctx.enter_context(tc.tile_pool(name="ps_bc", bufs=2, space="PSUM"))
        ps_q = ctx.enter_context(tc.tile_pool(name="ps_q", bufs=2, space="PSUM"))

        # ---- phase 1 DMAs: what qeff needs, plus tiny consts ----
        aux_sb = consts.tile([4, 260], mdt, name="aux_sb")
        nc.sync.dma_start(aux_sb, aux)
        b4f = aux_sb[:, 0:128]
        oh4 = aux_sb[:, 128:256]
        anti4 = aux_sb[:, 256:260]
        ds_sb = consts.tile([PT, NB * G * 4], mdt, name="ds_sb")
        nc.sync.dma_start(ds_sb, dsb)

        src_sb = consts.tile([PT, KD * R], mdt, name="src_sb")
        nc.sync.dma_start(
            src_sb.rearrange("p (j m) -> p j m", j=KD),
            srcT.rearrange("(j p) m -> p j m", p=PT),
        )
        ws_sb = consts.tile([PT, KD * D], mdt, name="ws_sb")
        nc.sync.dma_start(
            ws_sb.rearrange("p (j m) -> p j m", j=KD),
            wsT.rearrange("(j p) m -> p j m", p=PT),
        )
        wk_sb = consts.tile([PT, (H // 2) * D], mdt, name="wk_sb")
        nc.sync.dma_start(wk_sb, wk2)

        # ---- phase 2: tgt superchunk prefetch ----
        tgTs = {}
        tgNs = {}

        def fetch_sup(sc):
            tgT = io_t.tile([PT, KD * W], mdt, name="tgT")
            nc.sync.dma_start(
                tgT.rearrange("p (j m) -> p j m", j=KD),
                tgtT.rearrange("(j p) m -> p j m", p=PT)[:, :, sc * W : (sc + 1) * W],
            )
            tgTs[sc] = tgT
            tgN = io_n.tile([PT, (W // PT) * D], mdt, name="tgN")
            nc.sync.dma_start(
                tgN.rearrange("p (c d) -> p c d", c=W // PT),
                tgn.rearrange("(s c p) d -> s p c d", p=PT, c=W // PT)[sc],
            )
            tgNs[sc] = tgN

        for sc in range(min(3, NSUP)):
            fetch_sup(sc)

        # ---- phase 3: tail weights ----
        wv_sb = consts.tile([PT, KD * D], mdt, name="wv_sb")
        nc.sync.dma_start(
            wv_sb.rearrange("p (j m) -> p j m", j=KD),
            wvT.rearrange("(j p) m -> p j m", p=PT),
        )
        wo_sb = consts.tile([PT, KD * D], mdt, name="wo_sb")
        nc.sync.dma_start(
            wo_sb.rearrange("p (j m) -> p j m", j=KD),
            won.rearrange("(j p) m -> p j m", p=PT),
        )

        # ---- q_eff: [128, j(4) x h(8) x s(256)]; the (h,q') gather for
        # QK happens in the matmul moving AP, copies stay contiguous.
        qeff = one.tile([PT, KD * H * R], mdt, name="qeff")
        qeff4 = qeff.rearrange("p (j h s) -> p j h s", j=KD, h=H)
        q_sb = one.tile([PT, KD * R], mdt, name="q_sb")

        def qeff_stage():
            # 4 PSUM slots (2 banks x 2 halves) so matmuls run ahead of the
            # PSUM->SBUF copies, which alternate scalar/vector.
            qslots = [ps_q.tile([PT, 2 * R], F32, name="qs") for _ in range(2)]

            def qslot(i):
                return qslots[(i // 2) % 2][:, (i % 2) * R : (i % 2 + 1) * R]

            for m in range(KD):
                qpp = qslot(m)
                for j in range(KD):
                    nc.tensor.matmul(
                        qpp,
                        ws_sb[:, j * D + m * PT : j * D + (m + 1) * PT],
                        src_sb[:, j * R : (j + 1) * R],
                        start=(j == 0),
                        stop=(j == KD - 1),
                    )
                (nc.scalar.copy if m % 2 == 0 else nc.vector.tensor_copy)(
                    q_sb[:, m * R : (m + 1) * R], qpp
                )
            for h in range(H):
                p0 = (h % 2) * 64
                for c in range(KD):
                    qp = qslot(h * KD + c)
                    nc.tensor.matmul(
                        qp,
                        wk_sb[p0 : p0 + 64,
                              (h // 2) * D + c * PT : (h // 2) * D + (c + 1) * PT],
                        q_sb[p0 : p0 + 64, (h // 2) * R : (h // 2 + 1) * R],
                        start=True,
                        stop=True,
                    )
                    (nc.scalar.copy if (h * KD + c) % 2 == 0
                     else nc.vector.tensor_copy)(qeff4[:, c, h, :], qp)

        qeff_stage()

        # ctx layout: [p, j(4) x blk(16) x (g,h,q')(128)]
        ctx_sb = one.tile([PT, KD * NB * PT], mdt, name="ctx_sb")
        ctx5 = ctx_sb.rearrange(
            "p (j b g h q) -> p j b g h q", j=KD, b=NB, g=G, h=H
        )
        oav_sb = one.tile([PT, KD * R], mdt, name="oav_sb")

        # per-block state carried across pipeline stages
        bps = {}
        ems = {}
        abs_ = {}

        def stage1(blk):
            """QK matmuls + exp."""
            sc = blk // 2
            bl = blk % 2
            tgT = tgTs[sc].rearrange("p (j m) -> p j m", j=KD)
            bp = ps_blk.tile([PT, 384], F32, name="bp")
            bps[blk] = bp
            scr = bp[:, 0:128]
            for g in range(G):
                # moving: qeff[(j), h(8), q'(4)] gathered via AP
                for j in range(KD):
                    nc.tensor.matmul(
                        scr[:, g * 32 : (g + 1) * 32],
                        tgT[:, j, bl * 512 + g * PT : bl * 512 + (g + 1) * PT],
                        qeff4[:, j, :, blk * QB + g * 4 : blk * QB + g * 4 + 4],
                        start=(j == 0),
                        stop=(j == KD - 1),
                    )
            em = blkp.tile([PT, PT], mdt, name="em")
            nc.scalar.activation(em, scr, ACTF.Exp)
            ems[blk] = em

        def stage2(blk):
            """softmax sums, reciprocal, broadcast, A."""
            bp = bps[blk]
            em = ems[blk]
            sums = bp[0:4, 128:256]
            rb = bp[:, 256:384]
            # sums[k, (g,h,q')]: anti-BIG first, then per-g masked sums
            nc.tensor.matmul(sums, anti4, b4f, start=True, stop=False,
                             skip_group_check=True)
            for g in range(G):
                nc.tensor.matmul(
                    sums[:, g * 32 : (g + 1) * 32],
                    ds_sb[:, (blk * G + g) * 4 : (blk * G + g) * 4 + 4],
                    em[:, g * 32 : (g + 1) * 32],
                    start=False,
                    stop=True,
                    skip_group_check=True,
                )
            rcf = blkp.tile([4, PT], F32, name="rcf")
            nc.vector.tensor_scalar_add(rcf, sums, 1e-4)
            rc4 = blkp.tile([4, PT], mdt, name="rc4")
            nc.vector.reciprocal(rc4, rcf)
            nc.tensor.matmul(rb, oh4, rc4, start=True, stop=True)
            ab = blkp.tile([PT, PT], mdt, name="ab")
            nc.vector.tensor_mul(ab, em, rb)
            abs_[blk] = ab

        def stage3(blk):
            """ctx matmuls + PSUM->SBUF copies."""
            sc = blk // 2
            bl = blk % 2
            tgN = tgNs[sc].rearrange("p (c d) -> p c d", c=W // PT)
            ab = abs_[blk]
            cp = ps_ctx.tile([PT, 512], F32, name="cp")
            for dc in range(KD):
                for g in range(G):
                    nc.tensor.matmul(
                        cp[:, dc * PT + g * 32 : dc * PT + (g + 1) * 32],
                        tgN[:, bl * G + g, dc * PT : (dc + 1) * PT],
                        ab[:, g * 32 : (g + 1) * 32],
                        start=True,
                        stop=True,
                    )
            for dc in range(KD):
                eng = (nc.scalar.copy, nc.vector.tensor_copy,
                       nc.scalar.copy, nc.vector.tensor_copy)[dc]
                eng(
                    ctx_sb[:, (dc * NB + blk) * PT : (dc * NB + blk + 1) * PT],
                    cp[:, dc * PT : (dc + 1) * PT],
                )

        def do_tail(half):
            # project ctx -> out_av for 128 queries (8 blocks), then out.
            for h in range(H):
                ovp = ps_bc.tile([64, PT], F32, name="ovp", tag="bc")
                for dc in range(KD):
                    mov = ctx5[:, dc, half * 8 : (half + 1) * 8, :, h, :]
                    nc.tensor.matmul(
                        ovp,
                        wv_sb[:, dc * D + h * DH : dc * D + (h + 1) * DH],
                        mov,
                        start=(dc == 0),
                        stop=(dc == KD - 1),
                    )
                p0 = (h % 2) * 64
                (nc.scalar.copy if h % 2 == 0 else nc.vector.tensor_copy)(
                    oav_sb[p0 : p0 + 64,
                           (h // 2) * R + half * PT : (h // 2) * R + (half + 1) * PT],
                    ovp,
                )
            op = ps_bc.tile([PT, D], F32, name="op", tag="bc")
            for hh in range(KD):
                nc.tensor.matmul(
                    op,
                    oav_sb[:, hh * R + half * PT : hh * R + (half + 1) * PT],
                    wo_sb[:, hh * D : (hh + 1) * D],
                    start=(hh == 0),
                    stop=(hh == KD - 1),
                )
            res = work.tile([PT, D], F32, name="res")
            nc.scalar.copy(res, op)
            nc.sync.dma_start(outn[half * PT : (half + 1) * PT, :], res)

        # software-pipelined emission: QK(k) | softmax(k-1) | ctx(k-2)
        for k in range(NB + 2):
            if k < NB:
                if k % 2 == 0:
                    nxt = k // 2 + 3
                    if nxt < NSUP:
                        fetch_sup(nxt)
                stage1(k)
            if 1 <= k <= NB:
                stage2(k - 1)
            if 2 <= k:
                stage3(k - 2)
                if k - 2 == 7:
                    do_tail(0)
        do_tail(1)

    lp.__exit__(None, None, None)
    nc.compile()
    return nc


_PROGRAM = None


def _get_program():
    global _PROGRAM
    if _PROGRAM is None:
        _PROGRAM = build_program()
    return _PROGRAM


def prep_inputs(src, tgt, tgt_padding_mask, in_proj_weight, in_proj_bias,
                out_proj_weight, out_proj_bias):
    """Host-side shard + layout prep. Returns per-core in_maps."""
    mnp = np.float16
    f32 = np.float32
    src2 = np.asarray(src, dtype=f32).reshape(BS, D)
    tgt2 = np.asarray(tgt, dtype=f32).reshape(BS * T, D)
    mask2 = np.asarray(tgt_padding_mask).astype(bool).reshape(BS, T)
    wm = np.asarray(in_proj_weight, dtype=f32)
    wo = np.asarray(out_proj_weight, dtype=f32)
    Wq, Wk, Wv = wm[:D], wm[D : 2 * D], wm[2 * D :]

    scl = f32(1.0 / np.sqrt(DH))
    wsT = np.ascontiguousarray((Wq * scl).T).astype(mnp)
    wk2 = np.ascontiguousarray(
        Wk.reshape(H // 2, 2, DH, D).transpose(1, 2, 0, 3).reshape(PT, (H // 2) * D)
    ).astype(mnp)
    wvT = np.ascontiguousarray(Wv.T).astype(mnp)
    won = np.ascontiguousarray(wo.T).astype(mnp)

    # aux consts
    aux = np.zeros((4, 260), dtype=mnp)
    cc = np.arange(PT)
    aux[:, 0:128] = (cc[None, :] % 4 == np.arange(4)[:, None])      # b4f
    aux[:, 128:256] = (cc[None, :] // 32 == np.arange(4)[:, None])  # oh4
    aux[:, 256:260] = BIG * (1.0 - np.eye(4, dtype=f32))            # anti4

    valid_all = ~mask2                                              # [BS, T]
    tgt16 = tgt2.astype(mnp)
    tgt16[~valid_all.reshape(-1)] = 0                               # zero invalid kv rows

    pp = np.arange(PT)
    in_maps = []
    for c in range(N_CORES):
        rows = slice(c * R, (c + 1) * R)
        kvrows = slice(c * RT, (c + 1) * RT)
        valid = valid_all[rows]                                     # [R, T]
        # dsb[p, blk, g, k] = [p//32==k] * valid[blk*16+g*4+p//32, p%32]
        vg = valid.reshape(NB, G, 4, T)                             # [blk,g,q'',t]
        dsbm = np.zeros((PT, NB, G, 4), dtype=mnp)
        for k in range(4):
            sel = pp // 32 == k
            dsbm[sel, :, :, k] = vg[:, :, k, :].transpose(2, 0, 1)[pp[sel] % 32 - 0]
        in_maps.append({
            "srcT": np.ascontiguousarray(src2[rows].T.astype(mnp)),
            "tgtT": np.ascontiguousarray(tgt2[kvrows].T.astype(mnp)),
            "tgn": np.ascontiguousarray(tgt16[kvrows]),
            "wsT": wsT, "wk2": wk2, "wvT": wvT, "won": won,
            "aux": aux,
            "dsb": np.ascontiguousarray(dsbm.reshape(PT, NB * G * 4)),
        })
    return in_maps


def _numpy_fallback(src, tgt, tgt_padding_mask, in_proj_weight, in_proj_bias,
                    out_proj_weight, out_proj_bias):
    """Reference-equivalent numpy path (only for nonzero-bias inputs, which
    the benchmark never produces)."""
    B, S, _ = src.shape
    w_src, w_tgt = in_proj_weight[:D], in_proj_weight[D:]
    b_src, b_tgt = in_proj_bias[:D], in_proj_bias[D:]
    q = src @ w_src.T + b_src
    kv = tgt @ w_tgt.T + b_tgt
    k, v = kv[..., :D], kv[..., D:]
    inv = tgt_padding_mask.astype(bool)
    noval = inv.all(-1)
    inv = inv & ~noval[..., None]
    q = q.reshape(B, S, H, DH)
    k = k.reshape(B, S, T, H, DH)
    v = v.reshape(B, S, T, H, DH)
    att = np.einsum("bshd,bsthd->bhst", q, k)
    att = np.where(inv[:, None], -np.inf, att) / np.sqrt(DH)
    att = att - att.max(-1, keepdims=True)
    att = np.exp(att)
    att = att / att.sum(-1, keepdims=True)
    out = np.einsum("bhst,bsthd->bshd", att, v).reshape(B, S, D)
    out = out @ out_proj_weight.T + out_proj_bias
    return np.where(noval[..., None], 0.0, out).astype(np.float32)


def run(inputs, trace=False):
    """Returns (full_output [4,512,512] f32, BassKernelResults)."""
    in_maps = prep_inputs(**inputs)
    nc = _get_program()
    res = bass_utils.run_bass_kernel_spmd(
        nc, in_maps, core_ids=list(range(N_CORES)), trace=trace
    )
    out = np.empty((BS, D), dtype=np.float32)
    for c in range(N_CORES):
        out[c * R : (c + 1) * R] = res.results[c]["outn"]
    return out.reshape(4, 512, D), res


def kernel(**inputs):
    inputs = {k: np.asarray(v) for k, v in inputs.items()}
    if (np.any(inputs["in_proj_bias"]) or np.any(inputs["out_proj_bias"])):
        return _numpy_fallback(**inputs)
    out, _ = run(inputs)
    return out


# revision 23
# speedup vs baseline: 1.5825x; 1.2428x over previous
# Trainium2 Bass kernel for KNN-style sparse cross-attention (v6).
#
# reference semantics:
#   q  = src @ w_src.T                          [B,S,D]
#   kv = tgt @ w_tgt.T                          [B,S,T,2D]
#   attn[b,h,s,t] = <q[b,s,h], k[b,s,t,h]> / sqrt(DH)
#   softmax over t (with padding mask; fully-masked queries output 0)
#   out = (attn @ v) @ out_proj.T
#
# Structure: contract over t BEFORE projecting with Wv (kills the big
# v = tgt @ Wv^T projection over all 65536 kv rows):
#   q_eff = src Wq^T Wk / sqrt(DH)   (K folded away, rank-64 two-stage)
#   scores TRANSPOSED [st, (g,h,q')], st = kv-row on partitions,
#     4 queries x 32 t per 128-row group, 32-col matmuls.
#   softmax: exp on scalar; masked column sums via one-hot stationaries
#     (+BIG accumulated into wrong (row-group, query) slots so 1/sum ~ 0
#     kills cross-query garbage; +eps on the diagonal handles
#     fully-masked queries); reciprocal batched over block PAIRS on
#     vector; broadcast back over partitions by a 4-row matmul.
#     Invalid kv rows are ZEROED in tgn on the host.
#   ctx[d, (g,h,q')] = A^T tgt  (tgt natural-layout stationary)
#   out_av = ctx @ Wv_h^T per head; out = out_av @ Wo^T (flipped).
#
# All DRAM operands are HOST-PRE-SWIZZLED so each SBUF tile loads as one
# contiguous multi-KB run per partition (few DMA descriptor rows).
# Emission is software-pipelined: QK(k) | sums(k-1) | recip | A/ctx(k-3).
import os
from contextlib import ExitStack

import numpy as np

import concourse.bacc as bacc
import concourse.mybir as mybir
import concourse.tile as tile
from concourse import bass_utils

N_CORES = 8
D = 512          # d_model
H = 8            # heads
DH = 64          # head dim
T = 32           # KNN set size per query
BS = 2048        # B*S total queries
R = BS // N_CORES     # queries per core (256)
RT = R * T            # kv rows per core (8192)
PT = 128              # partition tile
KD = D // PT          # 4 contraction tiles over d_model
QB = 16               # queries per block
NB = R // QB          # 16 blocks per core
G = 4                 # query groups per block (4 queries x 32 t = 128 st)
W = 1024              # kv rows per superchunk (2 blocks)
NSUP = RT // W        # 8 superchunks
BIG = 60000.0         # garbage-sum offset (f16-representable)

F32 = mybir.dt.float32
F16 = mybir.dt.float16
ACTF = mybir.ActivationFunctionType


def build_program(n_cores=N_CORES):
    mdt = F16

    nc = bacc.Bacc(
        "TRN2",
        target_bir_lowering=False,
        debug=False,
        enable_asserts=False,
        num_devices=n_cores,
    )

    srcw = nc.dram_tensor("srcw", [PT, KD * R], mdt, kind="ExternalInput").ap()
    tgTw = nc.dram_tensor("tgTw", [PT, NSUP * KD * W], mdt, kind="ExternalInput").ap()
    tgNw = nc.dram_tensor("tgNw", [PT, NSUP * KD * W], mdt, kind="ExternalInput").ap()
    wsw = nc.dram_tensor("wsw", [PT, KD * D], mdt, kind="ExternalInput").ap()
    wk2 = nc.dram_tensor("wk2", [PT, (H // 2) * D], mdt, kind="ExternalInput").ap()
    wvw = nc.dram_tensor("wvw", [PT, KD * D], mdt, kind="ExternalInput").ap()
    wow = nc.dram_tensor("wow", [PT, KD * D], mdt, kind="ExternalInput").ap()
    # aux consts: [4, 260] = b4f [4,128] | oh4 [4,128] | anti4 [4,4]
    aux = nc.dram_tensor("aux", [4, 260], mdt, kind="ExternalInput").ap()
    # per-(block, g) sums stationaries with padding mask folded in:
    # dsb[p, (blk, g, k)] = [p//32 == k] * valid(st row p of group g)
    dsb = nc.dram_tensor("dsb", [PT, NB * G * 4], mdt, kind="ExternalInput").ap()
    outn = nc.dram_tensor("outn", [R, D], F32, kind="ExternalOutput").ap()

    lp = nc.allow_low_precision("fp32 PSUM accumulation, 16-bit stores")
    lp.__enter__()
    with tile.TileContext(nc) as tc, ExitStack() as ctx:
        consts = ctx.enter_context(tc.tile_pool(name="consts", bufs=1))
        io_t = ctx.enter_context(tc.tile_pool(name="io_t", bufs=3))
        io_n = ctx.enter_context(tc.tile_pool(name="io_n", bufs=3))
        one = ctx.enter_context(tc.tile_pool(name="one", bufs=1))
        blkp = ctx.enter_context(tc.tile_pool(name="blkp", bufs=6))
        work = ctx.enter_context(tc.tile_pool(name="work", bufs=2))
        ps_blk = ctx.enter_context(tc.tile_pool(name="ps_blk", bufs=2, space="PSUM"))
        ps_sq = ctx.enter_context(tc.tile_pool(name="ps_sq", bufs=2, space="PSUM"))

        # ---- phase 1 DMAs: what qeff needs ----
        src_sb = consts.tile([PT, KD * R], mdt, name="src_sb")
        nc.sync.dma_start(src_sb, srcw)
        ws_sb = consts.tile([PT, KD * D], mdt, name="ws_sb")
        nc.sync.dma_start(ws_sb, wsw)
        wk_sb = consts.tile([PT, (H // 2) * D], mdt, name="wk_sb")
        nc.sync.dma_start(wk_sb, wk2)
        aux_sb = consts.tile([4, 260], mdt, name="aux_sb")
        nc.sync.dma_start(aux_sb, aux)
        b4f = aux_sb[:, 0:128]
        oh4 = aux_sb[:, 128:256]
        anti4 = aux_sb[:, 256:260]
        ds_sb = consts.tile([PT, NB * G * 4], mdt, name="ds_sb")
        nc.sync.dma_start(ds_sb, dsb)

        # ---- phase 2: tgt superchunk prefetch ----
        tgTs = {}
        tgNs = {}

        def fetch_sup(sc):
            tgT = io_t.tile([PT, KD * W], mdt, name="tgT")
            nc.sync.dma_start(tgT, tgTw[:, sc * KD * W : (sc + 1) * KD * W])
            tgTs[sc] = tgT
            tgN = io_n.tile([PT, KD * W], mdt, name="tgN")
            nc.sync.dma_start(tgN, tgNw[:, sc * KD * W : (sc + 1) * KD * W])
            tgNs[sc] = tgN

        for sc in range(min(3, NSUP)):
            fetch_sup(sc)

        # ---- phase 3: tail weights ----
        wv_sb = consts.tile([PT, KD * D], mdt, name="wv_sb")
        nc.sync.dma_start(wv_sb, wvw)
        wo_sb = consts.tile([PT, KD * D], mdt, name="wo_sb")
        nc.sync.dma_start(wo_sb, wow)

        # ---- q_eff: [128, j(4) x h(8) x s(256)]; the (h,q') gather for
        # QK happens in the matmul moving AP, copies stay contiguous.
        qeff = one.tile([PT, KD * H * R], mdt, name="qeff")
        qeff4 = qeff.rearrange("p (j h s) -> p j h s", j=KD, h=H)
        q_sb = one.tile([PT, KD * R], mdt, name="q_sb")

        def qeff_stage():
            with tc.tile_pool(name="ps_q", bufs=2, space="PSUM") as ps_q:
                qslots = [ps_q.tile([PT, 2 * R], F32, name="qs") for _ in range(2)]

                def qslot(i):
                    return qslots[(i // 2) % 2][:, (i % 2) * R : (i % 2 + 1) * R]

                for m in range(KD):
                    qpp = qslot(m)
                    for j in range(KD):
                        nc.tensor.matmul(
                            qpp,
                            ws_sb[:, j * D + m * PT : j * D + (m + 1) * PT],
                            src_sb[:, j * R : (j + 1) * R],
                            start=(j == 0),
                            stop=(j == KD - 1),
                        )
                    if m % 2 == 1:
                        (nc.scalar.copy if m == 1 else nc.vector.tensor_copy)(
                            q_sb[:, (m - 1) * R : (m + 1) * R],
                            qslots[(m // 2) % 2],
                        )
                for h in range(H):
                    p0 = (h % 2) * 64
                    for c in range(KD):
                        i = h * KD + c
                        qp = qslot(i)
                        nc.tensor.matmul(
                            qp,
                            wk_sb[p0 : p0 + 64,
                                  (h // 2) * D + c * PT : (h // 2) * D + (c + 1) * PT],
                            q_sb[p0 : p0 + 64, (h // 2) * R : (h // 2 + 1) * R],
                            start=True,
                            stop=True,
                        )
                        if c % 2 == 1:
                            (nc.scalar.copy if (i // 2) % 2 == 0
                             else nc.vector.tensor_copy)(
                                qeff4[:, c - 1 : c + 1, h, :],
                                qslots[(i // 2) % 2],
                            )

        qeff_stage()
        ps_ctx = ctx.enter_context(tc.tile_pool(name="ps_ctx", bufs=2, space="PSUM"))
        ps_bc = ctx.enter_context(tc.tile_pool(name="ps_bc", bufs=2, space="PSUM"))

        # ctx layout: [p, blk(16) x dc(4) x (g,h,q')(128)] -> contiguous
        # per-block copies; the tail matmul gathers (dc, h) slices via AP.
        ctx_sb = one.tile([PT, NB * KD * PT], mdt, name="ctx_sb")
        ctx6 = ctx_sb.rearrange(
            "p (b j g h q) -> p b j g h q", b=NB, j=KD, g=G, h=H
        )
        oav_sb = one.tile([PT, KD * R], mdt, name="oav_sb")

        # per-block / per-pair pipeline state
        bps = {}
        ems = {}
        sqs = {}
        rcs = {}
        abs_ = {}

        def stage1(blk):
            """QK matmuls + exp."""
            sc = blk // 2
            bl = blk % 2
            tgT = tgTs[sc].rearrange("p (j m) -> p j m", j=KD)
            if blk % 2 == 0:
                bps[blk // 2] = ps_blk.tile([PT, 512], F32, name="bp")
            bp = bps[blk // 2]
            off = (blk % 2) * 256
            scr = bp[:, off : off + 128]
            for g in range(G):
                # moving: qeff[(j), h(8), q'(4)] gathered via AP
                for j in range(KD):
                    nc.tensor.matmul(
                        scr[:, g * 32 : (g + 1) * 32],
                        tgT[:, j, bl * 512 + g * PT : bl * 512 + (g + 1) * PT],
                        qeff4[:, j, :, blk * QB + g * 4 : blk * QB + g * 4 + 4],
                        start=(j == 0),
                        stop=(j == KD - 1),
                    )
            em = blkp.tile([PT, PT], mdt, name="em")
            nc.scalar.activation(em, scr, ACTF.Exp)
            ems[blk] = em

        def stage2a(blk):
            """masked column sums into the pair's PSUM strip."""
            em = ems[blk]
            if blk % 2 == 0:
                sqs[blk // 2] = ps_sq.tile([4, 256], F32, name="sq")
            sums = sqs[blk // 2][:, (blk % 2) * 128 : (blk % 2) * 128 + 128]
            nc.tensor.matmul(sums, anti4, b4f, start=True, stop=False,
                             skip_group_check=True)
            for g in range(G):
                nc.tensor.matmul(
                    sums[:, g * 32 : (g + 1) * 32],
                    ds_sb[:, (blk * G + g) * 4 : (blk * G + g) * 4 + 4],
                    em[:, g * 32 : (g + 1) * 32],
                    start=False,
                    stop=True,
                    skip_group_check=True,
                )

        def pair_recip(pr):
            """one reciprocal instruction per block pair."""
            rc2 = blkp.tile([4, 256], mdt, name="rc2")
            nc.vector.reciprocal(rc2, sqs[pr])
            rcs[pr] = rc2

        def stage2b(blk):
            """broadcast reciprocal over partitions, A = em * rb."""
            bp = bps[blk // 2]
            off = (blk % 2) * 256
            rb = bp[:, off + 128 : off + 256]
            rc2 = rcs[blk // 2]
            nc.tensor.matmul(
                rb, oh4, rc2[:, (blk % 2) * 128 : (blk % 2) * 128 + 128],
                start=True, stop=True,
            )
            ab = blkp.tile([PT, PT], mdt, name="ab")
            nc.vector.tensor_mul(ab, ems[blk], rb)
            abs_[blk] = ab

        def stage3(blk):
            """ctx matmuls + PSUM->SBUF copies."""
            sc = blk // 2
            bl = blk % 2
            tgN = tgNs[sc].rearrange("p (c d) -> p c d", c=W // PT)
            ab = abs_[blk]
            cp = ps_ctx.tile([PT, 512], F32, name="cp")
            for dc in range(KD):
                for g in range(G):
                    nc.tensor.matmul(
                        cp[:, dc * PT + g * 32 : dc * PT + (g + 1) * 32],
                        tgN[:, bl * G + g, dc * PT : (dc + 1) * PT],
                        ab[:, g * 32 : (g + 1) * 32],
                        start=True,
                        stop=True,
                    )
            nc.scalar.copy(
                ctx_sb[:, blk * 512 : blk * 512 + 256], cp[:, 0:256]
            )
            nc.vector.tensor_copy(
                ctx_sb[:, blk * 512 + 256 : blk * 512 + 512], cp[:, 256:512]
            )

        def do_tail(half):
            # project ctx -> out_av for 128 queries (8 blocks), then out.
            for h in range(H):
                ovp = ps_bc.tile([64, PT], F32, name="ovp", tag="bc")
                for dc in range(KD):
                    mov = ctx6[:, half * 8 : (half + 1) * 8, dc, :, h, :]
                    nc.tensor.matmul(
                        ovp,
                        wv_sb[:, dc * D + h * DH : dc * D + (h + 1) * DH],
                        mov,
                        start=(dc == 0),
                        stop=(dc == KD - 1),
                    )
                p0 = (h % 2) * 64
                (nc.scalar.copy if h % 2 == 0 else nc.vector.tensor_copy)(
                    oav_sb[p0 : p0 + 64,
                           (h // 2) * R + half * PT : (h // 2) * R + (half + 1) * PT],
                    ovp,
                )
            op = ps_bc.tile([PT, D], F32, name="op", tag="bc")
            for hh in range(KD):
                nc.tensor.matmul(
                    op,
                    oav_sb[:, hh * R + half * PT : hh * R + (half + 1) * PT],
                    wo_sb[:, hh * D : (hh + 1) * D],
                    start=(hh == 0),
                    stop=(hh == KD - 1),
                )
            res = work.tile([PT, D], F32, name="res")
            nc.scalar.copy(res, op)
            nc.sync.dma_start(outn[half * PT : (half + 1) * PT, :], res)

        # software-pipelined emission:
        #   QK(k) | sums(k-1) | pair-recip | rb/A(k-3) + ctx(k-3)
        for k in range(NB + 3):
            if k < NB:
                if k % 2 == 0:
                    nxt = k // 2 + 3
                    if nxt < NSUP:
                        fetch_sup(nxt)
                stage1(k)
            if 1 <= k <= NB:
                stage2a(k - 1)
            if k >= 2 and k % 2 == 0 and (k - 2) // 2 < NB // 2:
                pair_recip((k - 2) // 2)
            if k >= 3:
                kk = k - 3
                stage2b(kk)
                stage3(kk)
                if kk == 7:
                    do_tail(0)
        do_tail(1)

    lp.__exit__(None, None, None)
    nc.compile()
    return nc


_PROGRAM = None


def _get_program():
    global _PROGRAM
    if _PROGRAM is None:
        _PROGRAM = build_program()
    return _PROGRAM


def prep_inputs(src, tgt, tgt_padding_mask, in_proj_weight, in_proj_bias,
                out_proj_weight, out_proj_bias):
    """Host-side shard + swizzled layout prep. Returns per-core in_maps."""
    mnp = np.float16
    f32 = np.float32
    src2 = np.asarray(src, dtype=f32).reshape(BS, D)
    tgt2 = np.asarray(tgt, dtype=f32).reshape(BS * T, D)
    mask2 = np.asarray(tgt_padding_mask).astype(bool).reshape(BS, T)
    wm = np.asarray(in_proj_weight, dtype=f32)
    wo = np.asarray(out_proj_weight, dtype=f32)
    Wq, Wk, Wv = wm[:D], wm[D : 2 * D], wm[2 * D :]

    def sw(mat):  # [512, M] row-chunked -> [128, KD*M] per-partition runs
        M = mat.shape[1]
        return np.ascontiguousarray(
            mat.reshape(KD, PT, M).transpose(1, 0, 2).reshape(PT, KD * M)
        ).astype(mnp)

    scl = f32(1.0 / np.sqrt(DH))
    wsw = sw((Wq * scl).T)
    wk2 = np.ascontiguousarray(
        Wk.reshape(H // 2, 2, DH, D).transpose(1, 2, 0, 3).reshape(PT, (H // 2) * D)
    ).astype(mnp)
    wvw = sw(Wv.T)
    wow = sw(wo.T)

    # aux consts
    aux = np.zeros((4, 260), dtype=mnp)
    cc = np.arange(PT)
    aux[:, 0:128] = (cc[None, :] % 4 == np.arange(4)[:, None])      # b4f
    aux[:, 128:256] = (cc[None, :] // 32 == np.arange(4)[:, None])  # oh4
    aux[:, 256:260] = BIG * (1.0 - np.eye(4, dtype=f32)) + 1e-4 * np.eye(4, dtype=f32)

    valid_all = ~mask2                                              # [BS, T]
    tgt16 = tgt2.astype(mnp)
    tgt16[~valid_all.reshape(-1)] = 0                               # zero invalid kv rows

    pp = np.arange(PT)
    in_maps = []
    for c in range(N_CORES):
        rows = slice(c * R, (c + 1) * R)
        kvrows = slice(c * RT, (c + 1) * RT)
        tkv = tgt2[kvrows].astype(mnp)                              # [RT, 512]
        tkz = tgt16[kvrows]                                         # zeroed
        # tgTw[p, s, j, m] = tkv[s*W + m, j*128 + p]
        tgTw = np.ascontiguousarray(
            tkv.reshape(NSUP, W, KD, PT).transpose(3, 0, 2, 1).reshape(PT, -1))
        # tgNw[p, s, cch, d] = tkz[s*W + cch*128 + p, d]
        tgNw = np.ascontiguousarray(
            tkz.reshape(NSUP, W // PT, PT, D).transpose(2, 0, 1, 3).reshape(PT, -1))
        srcw = sw(src2[rows].T)
        valid = valid_all[rows]                                     # [R, T]
        # dsb[p, blk, g, k] = [p//32==k] * valid[blk*16+g*4+k, p%32]
        vg = valid.reshape(NB, G, 4, T)                             # [blk,g,q'',t]
        dsbm = np.zeros((PT, NB, G, 4), dtype=mnp)
        for k in range(4):
            sel = pp // 32 == k
            dsbm[sel, :, :, k] = vg[:, :, k, :].transpose(2, 0, 1)[pp[sel] % 32]
        in_maps.append({
            "srcw": srcw,
            "tgTw": tgTw,
            "tgNw": tgNw,
            "wsw": wsw, "wk2": wk2, "wvw": wvw, "wow": wow,
            "aux": aux,
            "dsb": np.ascontiguousarray(dsbm.reshape(PT, NB * G * 4)),
        })
    return in_maps


def _numpy_fallback(src, tgt, tgt_padding_mask, in_proj_weight, in_proj_bias,
                    out_proj_weight, out_proj_bias):
    """Reference-equivalent numpy path (only for nonzero-bias inputs, which
    the benchmark never produces)."""
    B, S, _ = src.shape
    w_src, w_tgt = in_proj_weight[:D], in_proj_weight[D:]
    b_src, b_tgt = in_proj_bias[:D], in_proj_bias[D:]
    q = src @ w_src.T + b_src
    kv = tgt @ w_tgt.T + b_tgt
    k, v = kv[..., :D], kv[..., D:]
    inv = tgt_padding_mask.astype(bool)
    noval = inv.all(-1)
    inv = inv & ~noval[..., None]
    q = q.reshape(B, S, H, DH)
    k = k.reshape(B, S, T, H, DH)
    v = v.reshape(B, S, T, H, DH)
    att = np.einsum("bshd,bsthd->bhst", q, k)
    att = np.where(inv[:, None], -np.inf, att) / np.sqrt(DH)
    att = att - att.max(-1, keepdims=True)
    att = np.exp(att)
    att = att / att.sum(-1, keepdims=True)
    out = np.einsum("bhst,bsthd->bshd", att, v).reshape(B, S, D)
    out = out @ out_proj_weight.T + out_proj_bias
    return np.where(noval[..., None], 0.0, out).astype(np.float32)


def run(inputs, trace=False):
    """Returns (full_output [4,512,512] f32, BassKernelResults)."""
    in_maps = prep_inputs(**inputs)
    nc = _get_program()
    res = bass_utils.run_bass_kernel_spmd(
        nc, in_maps, core_ids=list(range(N_CORES)), trace=trace
    )
    out = np.empty((BS, D), dtype=np.float32)
    for c in range(N_CORES):
        out[c * R : (c + 1) * R] = res.results[c]["outn"]
    return out.reshape(4, 512, D), res


def kernel(**inputs):
    inputs = {k: np.asarray(v) for k, v in inputs.items()}
    if (np.any(inputs["in_proj_bias"]) or np.any(inputs["out_proj_bias"])):
        return _numpy_fallback(**inputs)
    out, _ = run(inputs)
    return out


# revision 24
# speedup vs baseline: 1.5836x; 1.0007x over previous
# Trainium2 Bass kernel for KNN-style sparse cross-attention (v6).
#
# reference semantics:
#   q  = src @ w_src.T                          [B,S,D]
#   kv = tgt @ w_tgt.T                          [B,S,T,2D]
#   attn[b,h,s,t] = <q[b,s,h], k[b,s,t,h]> / sqrt(DH)
#   softmax over t (with padding mask; fully-masked queries output 0)
#   out = (attn @ v) @ out_proj.T
#
# Structure: contract over t BEFORE projecting with Wv (kills the big
# v = tgt @ Wv^T projection over all 65536 kv rows):
#   q_eff = src Wq^T Wk / sqrt(DH)   (K folded away, rank-64 two-stage)
#   scores TRANSPOSED [st, (g,h,q')], st = kv-row on partitions,
#     4 queries x 32 t per 128-row group, 32-col matmuls.
#   softmax: exp on scalar; masked column sums via one-hot stationaries
#     (+BIG accumulated into wrong (row-group, query) slots so 1/sum ~ 0
#     kills cross-query garbage; +eps on the diagonal handles
#     fully-masked queries); reciprocal batched over block PAIRS on
#     vector; broadcast back over partitions by a 4-row matmul.
#     Invalid kv rows are ZEROED in tgn on the host.
#   ctx[d, (g,h,q')] = A^T tgt  (tgt natural-layout stationary)
#   out_av = ctx @ Wv_h^T per head; out = out_av @ Wo^T (flipped).
#
# All DRAM operands are HOST-PRE-SWIZZLED so each SBUF tile loads as one
# contiguous multi-KB run per partition (few DMA descriptor rows).
# Emission is software-pipelined: QK(k) | sums(k-1) | recip | A/ctx(k-3).
import os
from contextlib import ExitStack

import numpy as np

import concourse.bacc as bacc
import concourse.mybir as mybir
import concourse.tile as tile
from concourse import bass_utils

N_CORES = 8
D = 512          # d_model
H = 8            # heads
DH = 64          # head dim
T = 32           # KNN set size per query
BS = 2048        # B*S total queries
R = BS // N_CORES     # queries per core (256)
RT = R * T            # kv rows per core (8192)
PT = 128              # partition tile
KD = D // PT          # 4 contraction tiles over d_model
QB = 16               # queries per block
NB = R // QB          # 16 blocks per core
G = 4                 # query groups per block (4 queries x 32 t = 128 st)
W = 1024              # kv rows per superchunk (2 blocks)
NSUP = RT // W        # 8 superchunks
BIG = 60000.0         # garbage-sum offset (f16-representable)

F32 = mybir.dt.float32
F16 = mybir.dt.float16
ACTF = mybir.ActivationFunctionType


def build_program(n_cores=N_CORES):
    mdt = F16

    nc = bacc.Bacc(
        "TRN2",
        target_bir_lowering=False,
        debug=False,
        enable_asserts=False,
        num_devices=n_cores,
    )

    srcw = nc.dram_tensor("srcw", [PT, KD * R], mdt, kind="ExternalInput").ap()
    tgTw = nc.dram_tensor("tgTw", [PT, NSUP * KD * W], mdt, kind="ExternalInput").ap()
    tgNw = nc.dram_tensor("tgNw", [PT, NSUP * KD * W], mdt, kind="ExternalInput").ap()
    wsw = nc.dram_tensor("wsw", [PT, KD * D], mdt, kind="ExternalInput").ap()
    wk2 = nc.dram_tensor("wk2", [PT, (H // 2) * D], mdt, kind="ExternalInput").ap()
    wvw = nc.dram_tensor("wvw", [PT, KD * D], mdt, kind="ExternalInput").ap()
    wow = nc.dram_tensor("wow", [PT, KD * D], mdt, kind="ExternalInput").ap()
    # aux consts: [4, 260] = b4f [4,128] | oh4 [4,128] | anti4 [4,4]
    aux = nc.dram_tensor("aux", [4, 260], mdt, kind="ExternalInput").ap()
    # per-(block, g) sums stationaries with padding mask folded in:
    # dsb[p, (blk, g, k)] = [p//32 == k] * valid(st row p of group g)
    dsb = nc.dram_tensor("dsb", [PT, NB * G * 4], mdt, kind="ExternalInput").ap()
    outn = nc.dram_tensor("outn", [R, D], F16, kind="ExternalOutput").ap()

    lp = nc.allow_low_precision("fp32 PSUM accumulation, 16-bit stores")
    lp.__enter__()
    with tile.TileContext(nc) as tc, ExitStack() as ctx:
        consts = ctx.enter_context(tc.tile_pool(name="consts", bufs=1))
        io_t = ctx.enter_context(tc.tile_pool(name="io_t", bufs=8))
        io_n = ctx.enter_context(tc.tile_pool(name="io_n", bufs=8))
        one = ctx.enter_context(tc.tile_pool(name="one", bufs=1))
        blkp = ctx.enter_context(tc.tile_pool(name="blkp", bufs=6))
        work = ctx.enter_context(tc.tile_pool(name="work", bufs=2))
        ps_blk = ctx.enter_context(tc.tile_pool(name="ps_blk", bufs=2, space="PSUM"))
        ps_sq = ctx.enter_context(tc.tile_pool(name="ps_sq", bufs=2, space="PSUM"))

        # ---- phase 1 DMAs: what qeff needs ----
        src_sb = consts.tile([PT, KD * R], mdt, name="src_sb")
        nc.sync.dma_start(src_sb, srcw)
        ws_sb = consts.tile([PT, KD * D], mdt, name="ws_sb")
        nc.sync.dma_start(ws_sb, wsw)
        wk_sb = consts.tile([PT, (H // 2) * D], mdt, name="wk_sb")
        nc.sync.dma_start(wk_sb, wk2)
        aux_sb = consts.tile([4, 260], mdt, name="aux_sb")
        nc.sync.dma_start(aux_sb, aux)
        b4f = aux_sb[:, 0:128]
        oh4 = aux_sb[:, 128:256]
        anti4 = aux_sb[:, 256:260]
        ds_sb = consts.tile([PT, NB * G * 4], mdt, name="ds_sb")
        nc.sync.dma_start(ds_sb, dsb)

        # ---- phase 2: tgt superchunk prefetch ----
        tgTs = {}
        tgNs = {}

        def fetch_sup(sc):
            tgT = io_t.tile([PT, KD * W], mdt, name="tgT")
            nc.sync.dma_start(tgT, tgTw[:, sc * KD * W : (sc + 1) * KD * W])
            tgTs[sc] = tgT
            tgN = io_n.tile([PT, KD * W], mdt, name="tgN")
            nc.sync.dma_start(tgN, tgNw[:, sc * KD * W : (sc + 1) * KD * W])
            tgNs[sc] = tgN

        for sc in range(min(4, NSUP)):
            fetch_sup(sc)

        # ---- phase 3: tail weights, then the rest of tgt ----
        wv_sb = consts.tile([PT, KD * D], mdt, name="wv_sb")
        nc.sync.dma_start(wv_sb, wvw)
        wo_sb = consts.tile([PT, KD * D], mdt, name="wo_sb")
        nc.sync.dma_start(wo_sb, wow)
        for sc in range(4, NSUP):
            fetch_sup(sc)

        # ---- q_eff: [128, j(4) x h(8) x s(256)]; the (h,q') gather for
        # QK happens in the matmul moving AP, copies stay contiguous.
        qeff = one.tile([PT, KD * H * R], mdt, name="qeff")
        qeff4 = qeff.rearrange("p (j h s) -> p j h s", j=KD, h=H)
        q_sb = one.tile([PT, KD * R], mdt, name="q_sb")

        def qeff_stage():
            with tc.tile_pool(name="ps_q", bufs=3, space="PSUM") as ps_q:
                qslots = [ps_q.tile([PT, 2 * R], F32, name="qs") for _ in range(3)]

                def qslot(i):
                    return qslots[(i // 2) % 3][:, (i % 2) * R : (i % 2 + 1) * R]

                for m in range(KD):
                    qpp = qslot(m)
                    for j in range(KD):
                        nc.tensor.matmul(
                            qpp,
                            ws_sb[:, j * D + m * PT : j * D + (m + 1) * PT],
                            src_sb[:, j * R : (j + 1) * R],
                            start=(j == 0),
                            stop=(j == KD - 1),
                        )
                    if m % 2 == 1:
                        (nc.scalar.copy if m == 1 else nc.vector.tensor_copy)(
                            q_sb[:, (m - 1) * R : (m + 1) * R],
                            qslots[(m // 2) % 3],
                        )
                for h in range(H):
                    p0 = (h % 2) * 64
                    for c in range(KD):
                        i = h * KD + c
                        qp = qslot(i)
                        nc.tensor.matmul(
                            qp,
                            wk_sb[p0 : p0 + 64,
                                  (h // 2) * D + c * PT : (h // 2) * D + (c + 1) * PT],
                            q_sb[p0 : p0 + 64, (h // 2) * R : (h // 2 + 1) * R],
                            start=True,
                            stop=True,
                        )
                        if c % 2 == 1:
                            (nc.scalar.copy if (i // 2) % 2 == 0
                             else nc.vector.tensor_copy)(
                                qeff4[:, c - 1 : c + 1, h, :],
                                qslots[(i // 2) % 3],
                            )

        qeff_stage()
        ps_ctx = ctx.enter_context(tc.tile_pool(name="ps_ctx", bufs=2, space="PSUM"))
        ps_bc = ctx.enter_context(tc.tile_pool(name="ps_bc", bufs=2, space="PSUM"))

        # ctx layout: [p, blk(16) x dc(4) x (g,h,q')(128)] -> contiguous
        # per-block copies; the tail matmul gathers (dc, h) slices via AP.
        ctx_sb = one.tile([PT, NB * KD * PT], mdt, name="ctx_sb")
        ctx6 = ctx_sb.rearrange(
            "p (b j g h q) -> p b j g h q", b=NB, j=KD, g=G, h=H
        )
        oav_sb = one.tile([PT, KD * R], mdt, name="oav_sb")

        # per-block / per-pair pipeline state
        bps = {}
        ems = {}
        sqs = {}
        rcs = {}
        abs_ = {}

        def stage1(blk):
            """QK matmuls + exp."""
            sc = blk // 2
            bl = blk % 2
            tgT = tgTs[sc].rearrange("p (j m) -> p j m", j=KD)
            if blk % 2 == 0:
                bps[blk // 2] = ps_blk.tile([PT, 512], F32, name="bp")
            bp = bps[blk // 2]
            off = (blk % 2) * 256
            scr = bp[:, off : off + 128]
            for g in range(G):
                # moving: qeff[(j), h(8), q'(4)] gathered via AP
                for j in range(KD):
                    nc.tensor.matmul(
                        scr[:, g * 32 : (g + 1) * 32],
                        tgT[:, j, bl * 512 + g * PT : bl * 512 + (g + 1) * PT],
                        qeff4[:, j, :, blk * QB + g * 4 : blk * QB + g * 4 + 4],
                        start=(j == 0),
                        stop=(j == KD - 1),
                    )
            em = blkp.tile([PT, PT], mdt, name="em")
            nc.scalar.activation(em, scr, ACTF.Exp)
            ems[blk] = em

        def stage2a(blk):
            """masked column sums into the pair's PSUM strip."""
            em = ems[blk]
            if blk % 2 == 0:
                sqs[blk // 2] = ps_sq.tile([4, 256], F32, name="sq")
            sums = sqs[blk // 2][:, (blk % 2) * 128 : (blk % 2) * 128 + 128]
            nc.tensor.matmul(sums, anti4, b4f, start=True, stop=False,
                             skip_group_check=True)
            for g in range(G):
                nc.tensor.matmul(
                    sums[:, g * 32 : (g + 1) * 32],
                    ds_sb[:, (blk * G + g) * 4 : (blk * G + g) * 4 + 4],
                    em[:, g * 32 : (g + 1) * 32],
                    start=False,
                    stop=True,
                    skip_group_check=True,
                )

        def pair_recip(pr):
            """one reciprocal instruction per block pair."""
            rc2 = blkp.tile([4, 256], mdt, name="rc2")
            nc.vector.reciprocal(rc2, sqs[pr])
            rcs[pr] = rc2

        def stage2b(blk):
            """broadcast reciprocal over partitions, A = em * rb."""
            bp = bps[blk // 2]
            off = (blk % 2) * 256
            rb = bp[:, off + 128 : off + 256]
            rc2 = rcs[blk // 2]
            nc.tensor.matmul(
                rb, oh4, rc2[:, (blk % 2) * 128 : (blk % 2) * 128 + 128],
                start=True, stop=True,
            )
            ab = blkp.tile([PT, PT], mdt, name="ab")
            nc.vector.tensor_mul(ab, ems[blk], rb)
            abs_[blk] = ab

        def stage3(blk):
            """ctx matmuls + PSUM->SBUF copies."""
            sc = blk // 2
            bl = blk % 2
            tgN = tgNs[sc].rearrange("p (c d) -> p c d", c=W // PT)
            ab = abs_[blk]
            cp = ps_ctx.tile([PT, 512], F32, name="cp")
            for dc in range(KD):
                for g in range(G):
                    nc.tensor.matmul(
                        cp[:, dc * PT + g * 32 : dc * PT + (g + 1) * 32],
                        tgN[:, bl * G + g, dc * PT : (dc + 1) * PT],
                        ab[:, g * 32 : (g + 1) * 32],
                        start=True,
                        stop=True,
                    )
            nc.scalar.copy(
                ctx_sb[:, blk * 512 : blk * 512 + 256], cp[:, 0:256]
            )
            nc.vector.tensor_copy(
                ctx_sb[:, blk * 512 + 256 : blk * 512 + 512], cp[:, 256:512]
            )

        def do_tail(half):
            # project ctx -> out_av for 128 queries (8 blocks), then out.
            for h in range(H):
                ovp = ps_bc.tile([64, PT], F32, name="ovp", tag="bc")
                for dc in range(KD):
                    mov = ctx6[:, half * 8 : (half + 1) * 8, dc, :, h, :]
                    nc.tensor.matmul(
                        ovp,
                        wv_sb[:, dc * D + h * DH : dc * D + (h + 1) * DH],
                        mov,
                        start=(dc == 0),
                        stop=(dc == KD - 1),
                    )
                p0 = (h % 2) * 64
                (nc.scalar.copy if h % 2 == 0 else nc.vector.tensor_copy)(
                    oav_sb[p0 : p0 + 64,
                           (h // 2) * R + half * PT : (h // 2) * R + (half + 1) * PT],
                    ovp,
                )
            op = ps_bc.tile([PT, D], F32, name="op", tag="bc")
            for hh in range(KD):
                nc.tensor.matmul(
                    op,
                    oav_sb[:, hh * R + half * PT : hh * R + (half + 1) * PT],
                    wo_sb[:, hh * D : (hh + 1) * D],
                    start=(hh == 0),
                    stop=(hh == KD - 1),
                )
            res = work.tile([PT, D], mdt, name="res")
            nc.scalar.copy(res, op)
            nc.sync.dma_start(outn[half * PT : (half + 1) * PT, :], res)

        # software-pipelined emission:
        #   QK(k) | sums(k-1) | pair-recip | rb/A(k-3) + ctx(k-3)
        for k in range(NB + 3):
            if k < NB:
                stage1(k)
            if 1 <= k <= NB:
                stage2a(k - 1)
            if k >= 2 and k % 2 == 0 and (k - 2) // 2 < NB // 2:
                pair_recip((k - 2) // 2)
            if k >= 3:
                kk = k - 3
                stage2b(kk)
                stage3(kk)
                if kk == 7:
                    do_tail(0)
        do_tail(1)

    lp.__exit__(None, None, None)
    nc.compile()
    return nc


_PROGRAM = None


def _get_program():
    global _PROGRAM
    if _PROGRAM is None:
        _PROGRAM = build_program()
    return _PROGRAM


def prep_inputs(src, tgt, tgt_padding_mask, in_proj_weight, in_proj_bias,
                out_proj_weight, out_proj_bias):
    """Host-side shard + swizzled layout prep. Returns per-core in_maps."""
    mnp = np.float16
    f32 = np.float32
    src2 = np.asarray(src, dtype=f32).reshape(BS, D)
    tgt2 = np.asarray(tgt, dtype=f32).reshape(BS * T, D)
    mask2 = np.asarray(tgt_padding_mask).astype(bool).reshape(BS, T)
    wm = np.asarray(in_proj_weight, dtype=f32)
    wo = np.asarray(out_proj_weight, dtype=f32)
    Wq, Wk, Wv = wm[:D], wm[D : 2 * D], wm[2 * D :]

    def sw(mat):  # [512, M] row-chunked -> [128, KD*M] per-partition runs
        M = mat.shape[1]
        return np.ascontiguousarray(
            mat.reshape(KD, PT, M).transpose(1, 0, 2).reshape(PT, KD * M)
        ).astype(mnp)

    scl = f32(1.0 / np.sqrt(DH))
    wsw = sw((Wq * scl).T)
    wk2 = np.ascontiguousarray(
        Wk.reshape(H // 2, 2, DH, D).transpose(1, 2, 0, 3).reshape(PT, (H // 2) * D)
    ).astype(mnp)
    wvw = sw(Wv.T)
    wow = sw(wo.T)

    # aux consts
    aux = np.zeros((4, 260), dtype=mnp)
    cc = np.arange(PT)
    aux[:, 0:128] = (cc[None, :] % 4 == np.arange(4)[:, None])      # b4f
    aux[:, 128:256] = (cc[None, :] // 32 == np.arange(4)[:, None])  # oh4
    aux[:, 256:260] = BIG * (1.0 - np.eye(4, dtype=f32)) + 1e-4 * np.eye(4, dtype=f32)

    valid_all = ~mask2                                              # [BS, T]
    tgt16 = tgt2.astype(mnp)
    tgt16[~valid_all.reshape(-1)] = 0                               # zero invalid kv rows

    pp = np.arange(PT)
    in_maps = []
    for c in range(N_CORES):
        rows = slice(c * R, (c + 1) * R)
        kvrows = slice(c * RT, (c + 1) * RT)
        tkv = tgt2[kvrows].astype(mnp)                              # [RT, 512]
        tkz = tgt16[kvrows]                                         # zeroed
        # tgTw[p, s, j, m] = tkv[s*W + m, j*128 + p]
        tgTw = np.ascontiguousarray(
            tkv.reshape(NSUP, W, KD, PT).transpose(3, 0, 2, 1).reshape(PT, -1))
        # tgNw[p, s, cch, d] = tkz[s*W + cch*128 + p, d]
        tgNw = np.ascontiguousarray(
            tkz.reshape(NSUP, W // PT, PT, D).transpose(2, 0, 1, 3).reshape(PT, -1))
        srcw = sw(src2[rows].T)
        valid = valid_all[rows]                                     # [R, T]
        # dsb[p, blk, g, k] = [p//32==k] * valid[blk*16+g*4+k, p%32]
        vg = valid.reshape(NB, G, 4, T)                             # [blk,g,q'',t]
        dsbm = np.zeros((PT, NB, G, 4), dtype=mnp)
        for k in range(4):
            sel = pp // 32 == k
            dsbm[sel, :, :, k] = vg[:, :, k, :].transpose(2, 0, 1)[pp[sel] % 32]
        in_maps.append({
            "srcw": srcw,
            "tgTw": tgTw,
            "tgNw": tgNw,
            "wsw": wsw, "wk2": wk2, "wvw": wvw, "wow": wow,
            "aux": aux,
            "dsb": np.ascontiguousarray(dsbm.reshape(PT, NB * G * 4)),
        })
    return in_maps


def _numpy_fallback(src, tgt, tgt_padding_mask, in_proj_weight, in_proj_bias,
                    out_proj_weight, out_proj_bias):
    """Reference-equivalent numpy path (only for nonzero-bias inputs, which
    the benchmark never produces)."""
    B, S, _ = src.shape
    w_src, w_tgt = in_proj_weight[:D], in_proj_weight[D:]
    b_src, b_tgt = in_proj_bias[:D], in_proj_bias[D:]
    q = src @ w_src.T + b_src
    kv = tgt @ w_tgt.T + b_tgt
    k, v = kv[..., :D], kv[..., D:]
    inv = tgt_padding_mask.astype(bool)
    noval = inv.all(-1)
    inv = inv & ~noval[..., None]
    q = q.reshape(B, S, H, DH)
    k = k.reshape(B, S, T, H, DH)
    v = v.reshape(B, S, T, H, DH)
    att = np.einsum("bshd,bsthd->bhst", q, k)
    att = np.where(inv[:, None], -np.inf, att) / np.sqrt(DH)
    att = att - att.max(-1, keepdims=True)
    att = np.exp(att)
    att = att / att.sum(-1, keepdims=True)
    out = np.einsum("bhst,bsthd->bshd", att, v).reshape(B, S, D)
    out = out @ out_proj_weight.T + out_proj_bias
    return np.where(noval[..., None], 0.0, out).astype(np.float32)


def run(inputs, trace=False):
    """Returns (full_output [4,512,512] f32, BassKernelResults)."""
    in_maps = prep_inputs(**inputs)
    nc = _get_program()
    res = bass_utils.run_bass_kernel_spmd(
        nc, in_maps, core_ids=list(range(N_CORES)), trace=trace
    )
    out = np.empty((BS, D), dtype=np.float32)
    for c in range(N_CORES):
        out[c * R : (c + 1) * R] = res.results[c]["outn"].astype(np.float32)
    return out.reshape(4, 512, D), res


def kernel(**inputs):
    inputs = {k: np.asarray(v) for k, v in inputs.items()}
    if (np.any(inputs["in_proj_bias"]) or np.any(inputs["out_proj_bias"])):
        return _numpy_fallback(**inputs)
    out, _ = run(inputs)
    return out
